# revision 1
# baseline (speedup 1.0000x reference)
"""Llama MHA layer on 8 TRN2 NeuronCores.

Sharding: causal-balanced sequence sharding, no collectives. Core c owns
batch-0 chunk c and batch-1 chunk 7-c (256 tokens each). Each core
recomputes K/V projections for its chunks' prefixes locally. Per-core
KV token columns are laid out [own | prefix | zero-pad] in two fixed-size
regions (1024 / 2048 cols) so the SPMD program is identical on all cores;
padding key-tiles are neutralized by a data-driven gate column fused into
the attention V matmul (which also computes the softmax denominator).

All activations are kept transposed ([feature, token]); matmuls run in
fp32r (full PE rate at free-dim>=256, ~2e-4 rms rounding). RoPE is done
in the transposed layout via a permutation matmul + two table multiplies;
rmsnorm uses a ones-column matmul for the cross-partition sum of squares
and a DRAM-bounce DMA for the partition broadcast of 1/rms.
"""

import sys
import types

import numpy as np

D_MODEL = 2048
N_HEADS = 32
N_KV = 8
HEAD_DIM = 64
D_FF = 8192
ROPE_BASE = 10000.0
EPS = 1e-5
B, S = 2, 2048
CHUNK = 256
P = 128
N_CORES = 8
R_SMALL = 1024   # cols in small-chunk region
R_LARGE = 2048   # cols in large-chunk region
N_KVCOL = R_SMALL + R_LARGE   # 3072
NEG = -1e30

_prog_cache = {}


def _build_program():
    import concourse.bacc as bacc
    import concourse.bass as bass
    import concourse.mybir as mybir
    import concourse.tile as tile

    F32 = mybir.dt.float32
    F32R = mybir.dt.float32r
    AF = mybir.ActivationFunctionType

    nc = bacc.Bacc(None, target_bir_lowering=False)

    # ---- inputs -------------------------------------------------------
    xT = nc.dram_tensor("xT", [D_MODEL, N_KVCOL], F32, kind="ExternalInput")
    cosT = nc.dram_tensor("cosT", [P, N_KVCOL], F32, kind="ExternalInput")
    sinT = nc.dram_tensor("sinT", [P, N_KVCOL], F32, kind="ExternalInput")
    maskd = nc.dram_tensor("maskd", [2, P, 256], F32, kind="ExternalInput")
    vgate = nc.dram_tensor("vgate", [2, 16, P], F32R, kind="ExternalInput")
    w_q = nc.dram_tensor("w_q", [D_MODEL, 2048], F32R, kind="ExternalInput")
    w_k = nc.dram_tensor("w_k", [D_MODEL, 512], F32R, kind="ExternalInput")
    w_v = nc.dram_tensor("w_v", [D_MODEL, 512], F32R, kind="ExternalInput")
    w_o = nc.dram_tensor("w_o", [D_MODEL, D_MODEL], F32R, kind="ExternalInput")
    w_g = nc.dram_tensor("w_g", [D_MODEL, D_FF], F32R, kind="ExternalInput")
    w_u = nc.dram_tensor("w_u", [D_MODEL, D_FF], F32R, kind="ExternalInput")
    w_d = nc.dram_tensor("w_d", [D_FF, D_MODEL], F32R, kind="ExternalInput")
    w_n1 = nc.dram_tensor("w_n1", [D_MODEL], F32, kind="ExternalInput")
    w_n2 = nc.dram_tensor("w_n2", [D_MODEL], F32, kind="ExternalInput")
    permM = nc.dram_tensor("permM", [P, P], F32R, kind="ExternalInput")
    onesC = nc.dram_tensor("onesC", [P, 1], F32R, kind="ExternalInput")
    outT = nc.dram_tensor("outT", [D_MODEL, 512], F32, kind="ExternalOutput")

    KD = D_MODEL // P       # 16 k-tiles over d_model
    BLK = 768               # phase-A column block
    NBLK = N_KVCOL // BLK   # 4
    INV_D = 1.0 / D_MODEL
    ATT_SCALE = 1.0 / np.sqrt(HEAD_DIM)

    _name_ctr = [0]

    def _nm(tag):
        _name_ctr[0] += 1
        return f"{tag}_{_name_ctr[0]}"

    def bcast_ap(dram_tile, parts, width, col0=0):
        return bass.AP(
            tensor=dram_tile.tensor,
            offset=dram_tile.offset + col0,
            ap=[[0, parts], [1, width]],
        )

    with tile.TileContext(nc) as tc:
        import contextlib
        stack = contextlib.ExitStack()
        with stack:
            dr = stack.enter_context(tc.tile_pool(name="dr", bufs=1, space="DRAM"))
            drb = stack.enter_context(tc.tile_pool(name="drb", bufs=4, space="DRAM"))
            const = stack.enter_context(tc.tile_pool(name="const", bufs=1))

            QT_s = dr.tile([2048, 512], F32R, tag="QT_s", name=_nm("QT_s"))
            KT_s = dr.tile([512, N_KVCOL], F32R, tag="KT_s", name=_nm("KT_s"))
            V_s = dr.tile([N_KVCOL, 512], F32R, tag="V_s", name=_nm("V_s"))

            ones_sb = const.tile([P, 1], F32R, tag="ones", name=_nm("ones"))
            nc.sync.dma_start(out=ones_sb, in_=onesC.ap())
            perm_sb = const.tile([P, P], F32R, tag="perm", name=_nm("perm"))
            nc.sync.dma_start(out=perm_sb, in_=permM.ap())
            w1_sb = const.tile([P, KD], F32, tag="w1", name=_nm("w1"))
            nc.sync.dma_start(out=w1_sb, in_=w_n1.ap().rearrange("(k p) -> p k", p=P))
            w2_sb = const.tile([P, KD], F32, tag="w2", name=_nm("w2"))
            nc.sync.dma_start(out=w2_sb, in_=w_n2.ap().rearrange("(k p) -> p k", p=P))
            mask_sb = const.tile([P, 2, 256], F32, tag="mask", name=_nm("mask"))
            nc.sync.dma_start(out=mask_sb, in_=maskd.ap().rearrange("t p n -> p t n"))
            eps_sb = const.tile([P, 1], F32, tag="eps", name=_nm("eps"))
            nc.vector.memset(eps_sb, EPS)

            # =========== PHASE A: rmsnorm1 + QKV proj + rope ===========
            with contextlib.ExitStack() as pa:
                xw_p = pa.enter_context(tc.tile_pool(name="xw", bufs=1))
                wv_p = pa.enter_context(tc.tile_pool(name="wv", bufs=1))
                tmp_p = pa.enter_context(tc.tile_pool(name="tmpA", bufs=3))
                wld_p = pa.enter_context(tc.tile_pool(name="wldA", bufs=4))
                tab_p = pa.enter_context(tc.tile_pool(name="tabA", bufs=2))
                col_p = pa.enter_context(tc.tile_pool(name="colA", bufs=8))
                rop_p = pa.enter_context(tc.tile_pool(name="ropA", bufs=3))
                ps_mm = pa.enter_context(
                    tc.tile_pool(name="psmmA", bufs=4, space="PSUM"))
                ps_st = pa.enter_context(
                    tc.tile_pool(name="psstA", bufs=1, space="PSUM"))
                ps_rp = pa.enter_context(
                    tc.tile_pool(name="psrpA", bufs=1, space="PSUM"))

                # w_v fully resident (reused by all 24 token-tiles)
                wv_sb = wv_p.tile([P, KD, 512], F32R, tag="wv", name=_nm("wv"))
                nc.sync.dma_start(
                    out=wv_sb, in_=w_v.ap().rearrange("(k p) n -> p k n", p=P))

                for blk in range(NBLK):
                    c0 = blk * BLK
                    xw = xw_p.tile([P, KD, BLK], F32R, tag="xw", name=_nm("xw"))
                    ssum = ps_st.tile([1, BLK], F32, tag="ss", name=_nm("ss"))
                    for k in range(KD):
                        xr = tmp_p.tile([P, BLK], F32, tag="xr", name=_nm("xr"))
                        nc.sync.dma_start(
                            out=xr, in_=xT.ap()[k * P:(k + 1) * P, c0:c0 + BLK])
                        sq = tmp_p.tile([P, BLK], F32R, tag="sq", name=_nm("sq"))
                        nc.vector.tensor_mul(sq[:], xr[:], xr[:])
                        for s0, sw in ((0, 512), (512, 256)):
                            nc.tensor.matmul(
                                ssum[:, s0:s0 + sw], ones_sb[:],
                                sq[:, s0:s0 + sw],
                                start=(k == 0), stop=(k == KD - 1))
                        nc.vector.tensor_scalar_mul(
                            xw[:, k, :], xr[:], w1_sb[:, k:k + 1])

                    # inv rms -> DRAM bounce -> broadcast + per-token cols
                    std_r = tmp_p.tile([1, BLK], F32, tag="std", name=_nm("std"))
                    nc.scalar.activation(
                        std_r[:], ssum[:], AF.Sqrt, bias=eps_sb[0:1, :], scale=INV_D)
                    inv_r = tmp_p.tile([1, BLK], F32, tag="inv", name=_nm("inv"))
                    nc.vector.reciprocal(inv_r[:], std_r[:])
                    bnc = drb.tile([1, BLK], F32, tag="bncA", name=_nm("bncA"))
                    nc.sync.dma_start(out=bnc[:], in_=inv_r)
                    ibc = tab_p.tile([P, BLK], F32, tag="ibc", name=_nm("ibc"))
                    nc.sync.dma_start(out=ibc, in_=bcast_ap(bnc, P, BLK))
                    invcols = []
                    for tm in range(BLK // P):
                        icol = col_p.tile([P, 1], F32, tag="icol", name=_nm("icol"))
                        nc.sync.dma_start(
                            out=icol,
                            in_=bass.AP(tensor=bnc.tensor,
                                        offset=bnc.offset + tm * P,
                                        ap=[[1, P], [1, 1]]))
                        invcols.append(icol)

                    # scaled rope tables (inv_rms folded in)
                    cosS = tab_p.tile([P, BLK], F32, tag="cosS", name=_nm("cosS"))
                    craw = tmp_p.tile([P, BLK], F32, tag="craw", name=_nm("craw"))
                    nc.sync.dma_start(
                        out=craw, in_=cosT.ap()[:, c0:c0 + BLK])
                    nc.vector.tensor_mul(cosS[:], craw[:], ibc[:])
                    sinS = tab_p.tile([P, BLK], F32, tag="sinS", name=_nm("sinS"))
                    sraw = tmp_p.tile([P, BLK], F32, tag="sraw", name=_nm("sraw"))
                    nc.sync.dma_start(
                        out=sraw, in_=sinT.ap()[:, c0:c0 + BLK])
                    nc.vector.tensor_mul(sinS[:], sraw[:], ibc[:])

                    def rope_spill(psum, w, cos_ap, sin_ap, dst_ap):
                        raw = rop_p.tile([P, 512], F32R, tag="rraw", name=_nm("rraw"))[:, :w]
                        nc.scalar.activation(raw, psum, AF.Copy)
                        rot = ps_rp.tile([P, 512], F32, tag="rot", name=_nm("rot"))[:, :w]
                        nc.tensor.matmul(rot, perm_sb[:], raw,
                                         start=True, stop=True)
                        t1 = rop_p.tile([P, 512], F32, tag="t1", name=_nm("t1"))[:, :w]
                        nc.vector.tensor_mul(t1, raw, cos_ap)
                        t2 = rop_p.tile([P, 512], F32, tag="t2", name=_nm("t2"))[:, :w]
                        nc.vector.tensor_mul(t2, rot, sin_ap)
                        fin = rop_p.tile([P, 512], F32R, tag="fin", name=_nm("fin"))[:, :w]
                        nc.vector.tensor_add(fin, t1, t2)
                        nc.sync.dma_start(out=dst_ap, in_=fin)

                    # K projection: KT_s[512, cols] (+rope)
                    for mg in range(2):
                        for s0, sw in ((0, 512), (512, 256)):
                            kps = [ps_mm.tile([P, 512], F32, tag="mm", name=_nm("mm"))[:, :sw]
                                   for _ in range(2)]
                            for k in range(KD):
                                wk_t = wld_p.tile([P, 256], F32R, tag="wld", name=_nm("wld"))
                                nc.sync.dma_start(
                                    out=wk_t,
                                    in_=w_k.ap()[k * P:(k + 1) * P,
                                                 mg * 256:(mg + 1) * 256])
                                for mi in range(2):
                                    nc.tensor.matmul(
                                        kps[mi],
                                        wk_t[:, mi * P:(mi + 1) * P],
                                        xw[:, k, s0:s0 + sw],
                                        start=(k == 0), stop=(k == KD - 1))
                            for mi in range(2):
                                m = mg * 2 + mi
                                rope_spill(
                                    kps[mi], sw,
                                    cosS[:, s0:s0 + sw], sinS[:, s0:s0 + sw],
                                    KT_s[m * P:(m + 1) * P,
                                         c0 + s0:c0 + s0 + sw])

                    # V projection: V_s[cols, 512] (inv_rms via ACT scale)
                    for tm in range(BLK // P):
                        vps = ps_mm.tile([P, 512], F32, tag="mm", name=_nm("mm"))
                        for k in range(KD):
                            nc.tensor.matmul(
                                vps[:], xw[:, k, tm * P:(tm + 1) * P],
                                wv_sb[:, k, :],
                                start=(k == 0), stop=(k == KD - 1))
                        vt = tmp_p.tile([P, 512], F32R, tag="vt", name=_nm("vt"))
                        nc.scalar.activation(
                            vt[:], vps[:], AF.Copy, scale=invcols[tm][:])
                        nc.sync.dma_start(
                            out=V_s[c0 + tm * P:c0 + (tm + 1) * P, :], in_=vt)

                    # Q projection (blocks 0/1 only hold own columns)
                    if blk in (0, 1):
                        os_ = 0 if blk == 0 else 256  # own cols inside block
                        q0 = 0 if blk == 0 else 256   # dst col in QT_s
                        for mg in range(8):
                            qps = [ps_mm.tile([P, 512], F32, tag="mm", name=_nm("mm"))[:, :256]
                                   for _ in range(2)]
                            for k in range(KD):
                                wq_t = wld_p.tile([P, 256], F32R, tag="wld", name=_nm("wld"))
                                nc.sync.dma_start(
                                    out=wq_t,
                                    in_=w_q.ap()[k * P:(k + 1) * P,
                                                 mg * 256:(mg + 1) * 256])
                                for mi in range(2):
                                    nc.tensor.matmul(
                                        qps[mi],
                                        wq_t[:, mi * P:(mi + 1) * P],
                                        xw[:, k, os_:os_ + 256],
                                        start=(k == 0), stop=(k == KD - 1))
                            for mi in range(2):
                                m = mg * 2 + mi
                                rope_spill(
                                    qps[mi], 256,
                                    cosS[:, os_:os_ + 256],
                                    sinS[:, os_:os_ + 256],
                                    QT_s[m * P:(m + 1) * P, q0:q0 + 256])

            # =========== PHASE B: attention ===========
            ctx_p = stack.enter_context(tc.tile_pool(name="ctx", bufs=1))
            ctxt = ctx_p.tile([P, KD, 512], F32R, tag="ctxt", name=_nm("ctxt"))
            with contextlib.ExitStack() as pb:
                kv_p = pb.enter_context(tc.tile_pool(name="kvB", bufs=2))
                va_p = pb.enter_context(tc.tile_pool(name="vaB", bufs=20))
                qh_p = pb.enter_context(tc.tile_pool(name="qhB", bufs=4))
                ex_p = pb.enter_context(tc.tile_pool(name="exB", bufs=6))
                sm_p = pb.enter_context(tc.tile_pool(name="smB", bufs=6))
                ps_sc = pb.enter_context(
                    tc.tile_pool(name="pssc", bufs=4, space="PSUM"))
                ps_cx = pb.enter_context(
                    tc.tile_pool(name="pscx", bufs=2, space="PSUM"))

                for g in range(N_KV):
                    for cc in range(2):
                        nkt = 8 if cc == 0 else 16
                        kc0 = 0 if cc == 0 else R_SMALL
                        ksb = kv_p.tile([64, 2048], F32R, tag="ksb", name=_nm("ksb"))
                        nc.sync.dma_start(
                            out=ksb[:, :nkt * P],
                            in_=KT_s[g * 64:(g + 1) * 64, kc0:kc0 + nkt * P])
                        vaugs = []
                        for kt in range(nkt):
                            va = va_p.tile([P, 65], F32R, tag="va", name=_nm("va"))
                            nc.sync.dma_start(
                                out=va[:, 0:64],
                                in_=V_s[kc0 + kt * P:kc0 + (kt + 1) * P,
                                        g * 64:(g + 1) * 64])
                            nc.sync.dma_start(
                                out=va[:, 64:65],
                                in_=vgate.ap()[cc, kt, :].rearrange(
                                    "(p o) -> p o", o=1))
                            vaugs.append(va)
                        for h4 in range(4):
                            h = g * 4 + h4
                            qh = qh_p.tile([64, 256], F32R, tag="qh", name=_nm("qh"))
                            nc.sync.dma_start(
                                out=qh,
                                in_=QT_s[h * 64:(h + 1) * 64,
                                         cc * 256:(cc + 1) * 256])
                            cxp = ps_cx.tile([65, 256], F32, tag="cx", name=_nm("cx"))
                            for kt in range(nkt):
                                scp = ps_sc.tile([P, 256], F32, tag="sc", name=_nm("sc"))
                                nc.tensor.matmul(
                                    scp[:], ksb[:, kt * P:(kt + 1) * P],
                                    qh[:], start=True, stop=True)
                                if kt < 2:
                                    nc.vector.tensor_add(
                                        scp[:], scp[:], mask_sb[:, kt, :])
                                ex = ex_p.tile([P, 256], F32R, tag="ex", name=_nm("ex"))
                                nc.scalar.activation(
                                    ex[:], scp[:], AF.Exp, scale=ATT_SCALE)
                                nc.tensor.matmul(
                                    cxp[:], vaugs[kt][:], ex[:],
                                    start=(kt == 0), stop=(kt == nkt - 1))
                            rec = sm_p.tile([1, 256], F32, tag="rec", name=_nm("rec"))
                            nc.vector.reciprocal(rec[:], cxp[64:65, :])
                            bn2 = drb.tile([1, 256], F32, tag="bncB", name=_nm("bncB"))
                            nc.sync.dma_start(out=bn2[:], in_=rec)
                            bcr = sm_p.tile([64, 256], F32, tag="bcr", name=_nm("bcr"))
                            nc.sync.dma_start(out=bcr, in_=bcast_ap(bn2, 64, 256))
                            nc.vector.tensor_mul(
                                ctxt[(h % 2) * 64:(h % 2) * 64 + 64, h // 2,
                                     cc * 256:(cc + 1) * 256],
                                cxp[0:64, :], bcr[:])

            # =========== PHASE C: out-proj + residual + rmsnorm2 =======
            res_p = stack.enter_context(tc.tile_pool(name="res", bufs=1))
            h2_p = stack.enter_context(tc.tile_pool(name="h2", bufs=1))
            yT = res_p.tile([P, KD, 512], F32, tag="yT", name=_nm("yT"))
            h2 = h2_p.tile([P, KD, 512], F32R, tag="h2", name=_nm("h2"))
            with contextlib.ExitStack() as pc:
                xo_p = pc.enter_context(tc.tile_pool(name="xoC", bufs=1))
                wld2_p = pc.enter_context(tc.tile_pool(name="wldC", bufs=4))
                tmp2_p = pc.enter_context(tc.tile_pool(name="tmpC", bufs=4))
                ps_y = pc.enter_context(
                    tc.tile_pool(name="psyC", bufs=2, space="PSUM"))
                ps_s2 = pc.enter_context(
                    tc.tile_pool(name="pss2", bufs=1, space="PSUM"))

                xo = xo_p.tile([P, KD, 512], F32, tag="xo", name=_nm("xo"))
                for k in range(KD):
                    nc.sync.dma_start(
                        out=xo[:, k, 0:256], in_=xT.ap()[k * P:(k + 1) * P, 0:256])
                    nc.sync.dma_start(
                        out=xo[:, k, 256:512],
                        in_=xT.ap()[k * P:(k + 1) * P, R_SMALL:R_SMALL + 256])

                for mg in range(8):
                    yps = [ps_y.tile([P, 512], F32, tag="y", name=_nm("y")) for _ in range(2)]
                    for k in range(KD):
                        wo_t = wld2_p.tile([P, 256], F32R, tag="wld", name=_nm("wld"))
                        nc.sync.dma_start(
                            out=wo_t,
                            in_=w_o.ap()[k * P:(k + 1) * P,
                                         mg * 256:(mg + 1) * 256])
                        for mi in range(2):
                            nc.tensor.matmul(
                                yps[mi], wo_t[:, mi * P:(mi + 1) * P],
                                ctxt[:, k, :],
                                start=(k == 0), stop=(k == KD - 1))
                    for mi in range(2):
                        m = mg * 2 + mi
                        nc.vector.tensor_add(yT[:, m, :], yps[mi][:], xo[:, m, :])

                ss2 = ps_s2.tile([1, 512], F32, tag="ss2", name=_nm("ss2"))
                for m in range(KD):
                    sq2 = tmp2_p.tile([P, 512], F32R, tag="sq2", name=_nm("sq2"))
                    nc.vector.tensor_mul(sq2[:], yT[:, m, :], yT[:, m, :])
                    nc.tensor.matmul(ss2[:], ones_sb[:], sq2[:],
                                     start=(m == 0), stop=(m == KD - 1))
                std2 = tmp2_p.tile([1, 512], F32, tag="std2", name=_nm("std2"))
                nc.scalar.activation(std2[:], ss2[:], AF.Sqrt,
                                     bias=eps_sb[0:1, :], scale=INV_D)
                inv2 = tmp2_p.tile([1, 512], F32, tag="inv2", name=_nm("inv2"))
                nc.vector.reciprocal(inv2[:], std2[:])
                bn3 = drb.tile([1, 512], F32, tag="bncC", name=_nm("bncC"))
                nc.sync.dma_start(out=bn3[:], in_=inv2)
                ibc2 = xo_p.tile([P, 512], F32, tag="ibc2", name=_nm("ibc2"))
                nc.sync.dma_start(out=ibc2, in_=bcast_ap(bn3, P, 512))
                for m in range(KD):
                    t = tmp2_p.tile([P, 512], F32, tag="tC", name=_nm("tC"))
                    nc.vector.tensor_mul(t[:], yT[:, m, :], ibc2[:])
                    nc.vector.tensor_scalar_mul(h2[:, m, :], t[:], w2_sb[:, m:m + 1])

            # =========== PHASE D: SwiGLU MLP ===========
            with contextlib.ExitStack() as pd:
                ht_p = pd.enter_context(tc.tile_pool(name="htD", bufs=18))
                y2_p = pd.enter_context(tc.tile_pool(name="y2D", bufs=1))
                wld3_p = pd.enter_context(tc.tile_pool(name="wldD", bufs=6))
                tmp3_p = pd.enter_context(tc.tile_pool(name="tmpD", bufs=4))
                ps_gu = pd.enter_context(
                    tc.tile_pool(name="psgu", bufs=4, space="PSUM"))
                ps_d = pd.enter_context(
                    tc.tile_pool(name="psd", bufs=2, space="PSUM"))

                y2acc = y2_p.tile([P, KD, 512], F32, tag="y2", name=_nm("y2"))
                for grp in range(4):
                    f0 = grp * 2048
                    hts = []
                    for fg in range(8):
                        gps = [ps_gu.tile([P, 512], F32, tag="gu", name=_nm("gu"))
                               for _ in range(2)]
                        ups = [ps_gu.tile([P, 512], F32, tag="gu", name=_nm("gu"))
                               for _ in range(2)]
                        for k in range(KD):
                            wg_t = wld3_p.tile([P, 256], F32R, tag="wld", name=_nm("wld"))
                            nc.sync.dma_start(
                                out=wg_t,
                                in_=w_g.ap()[k * P:(k + 1) * P,
                                             f0 + fg * 256:f0 + (fg + 1) * 256])
                            wu_t = wld3_p.tile([P, 256], F32R, tag="wld", name=_nm("wld"))
                            nc.sync.dma_start(
                                out=wu_t,
                                in_=w_u.ap()[k * P:(k + 1) * P,
                                             f0 + fg * 256:f0 + (fg + 1) * 256])
                            for mi in range(2):
                                nc.tensor.matmul(
                                    gps[mi], wg_t[:, mi * P:(mi + 1) * P],
                                    h2[:, k, :],
                                    start=(k == 0), stop=(k == KD - 1))
                                nc.tensor.matmul(
                                    ups[mi], wu_t[:, mi * P:(mi + 1) * P],
                                    h2[:, k, :],
                                    start=(k == 0), stop=(k == KD - 1))
                        for mi in range(2):
                            sil = tmp3_p.tile([P, 512], F32, tag="sil", name=_nm("sil"))
                            nc.scalar.activation(sil[:], gps[mi][:], AF.Silu)
                            ht = ht_p.tile([P, 512], F32R, tag="ht", name=_nm("ht"))
                            nc.vector.tensor_mul(ht[:], sil[:], ups[mi][:])
                            hts.append(ht)
                    for mg in range(8):
                        dps = [ps_d.tile([P, 512], F32, tag="d", name=_nm("d"))
                               for _ in range(2)]
                        for kk in range(16):
                            kr = (f0 + kk * P)
                            wd_t = wld3_p.tile([P, 256], F32R, tag="wld", name=_nm("wld"))
                            nc.sync.dma_start(
                                out=wd_t,
                                in_=w_d.ap()[kr:kr + P,
                                             mg * 256:(mg + 1) * 256])
                            for mi in range(2):
                                nc.tensor.matmul(
                                    dps[mi], wd_t[:, mi * P:(mi + 1) * P],
                                    hts[kk][:],
                                    start=(kk == 0), stop=(kk == 15))
                        for mi in range(2):
                            m = mg * 2 + mi
                            if grp == 0:
                                nc.vector.tensor_copy(y2acc[:, m, :], dps[mi][:])
                            else:
                                nc.vector.tensor_add(
                                    y2acc[:, m, :], y2acc[:, m, :], dps[mi][:])

                for m in range(KD):
                    o = tmp3_p.tile([P, 512], F32, tag="o", name=_nm("o"))
                    nc.vector.tensor_add(o[:], y2acc[:, m, :], yT[:, m, :])
                    nc.sync.dma_start(
                        out=outT.ap()[m * P:(m + 1) * P, :], in_=o)

    nc.compile()
    return nc


# ======================= host-side prep =======================

def _host_prep(c, x, w_norm1, w_qkv, w_out, w_norm2, w_gate, w_up, w_down):
    """Build the per-core input map (numpy only, layout/slicing + tables)."""
    f32 = np.float32
    if c <= 3:
        b_small, ch_small = 0, c
        b_large, ch_large = 1, 7 - c
    else:
        b_small, ch_small = 1, 7 - c
        b_large, ch_large = 0, c

    xT_full0 = x[b_small].T  # [D, S]
    xT_full1 = x[b_large].T

    xTc = np.zeros((D_MODEL, N_KVCOL), dtype=f32)
    pos = np.zeros(N_KVCOL, dtype=np.int64)
    # small region: [own | prefix | pad]
    o0 = ch_small * CHUNK
    xTc[:, 0:CHUNK] = xT_full0[:, o0:o0 + CHUNK]
    pos[0:CHUNK] = np.arange(o0, o0 + CHUNK)
    npre = o0
    xTc[:, CHUNK:CHUNK + npre] = xT_full0[:, 0:npre]
    pos[CHUNK:CHUNK + npre] = np.arange(npre)
    # large region
    o1 = ch_large * CHUNK
    xTc[:, R_SMALL:R_SMALL + CHUNK] = xT_full1[:, o1:o1 + CHUNK]
    pos[R_SMALL:R_SMALL + CHUNK] = np.arange(o1, o1 + CHUNK)
    npre1 = o1
    xTc[:, R_SMALL + CHUNK:R_SMALL + CHUNK + npre1] = xT_full1[:, 0:npre1]
    pos[R_SMALL + CHUNK:R_SMALL + CHUNK + npre1] = np.arange(npre1)

    # rope tables, replicated for 2 heads per 128 partitions, sign folded
    inv_freq = (ROPE_BASE ** (-np.arange(0, HEAD_DIM, 2, dtype=np.float64)
                              / HEAD_DIM))  # [32]
    ang = pos[None, :] * inv_freq[:, None]          # [32, N_KVCOL]
    cos32 = np.cos(ang)
    sin32 = np.sin(ang)
    cosT = np.empty((P, N_KVCOL), dtype=f32)
    sinT = np.empty((P, N_KVCOL), dtype=f32)
    for hh in range(2):
        r = hh * 64
        cosT[r:r + 32] = cos32
        cosT[r + 32:r + 64] = cos32
        sinT[r:r + 32] = -sin32
        sinT[r + 32:r + 64] = sin32

    # diagonal causal masks (key idx kt*128+k vs query idx j)
    maskd = np.zeros((2, P, 256), dtype=f32)
    j = np.arange(256)[None, :]
    k_ = np.arange(P)[:, None]
    maskd[0] = np.where(k_ > j, NEG, 0.0)
    maskd[1] = np.where(k_ + P > j, NEG, 0.0)

    # gate column: 1.0 for real key-tiles, 0.0 for padding
    vgate = np.zeros((2, 16, P), dtype=f32)
    vgate[0, :2 + 2 * ch_small, :] = 1.0
    vgate[1, :2 + 2 * ch_large, :] = 1.0

    perm = np.zeros((P, P), dtype=f32)
    for r in range(P):
        d = r % 64
        s = r + 32 if d < 32 else r - 32
        perm[s, r] = 1.0

    return {
        "xT": np.ascontiguousarray(xTc),
        "cosT": cosT, "sinT": sinT, "maskd": maskd, "vgate": vgate,
        "w_q": np.ascontiguousarray(w_qkv[:, :2048]),
        "w_k": np.ascontiguousarray(w_qkv[:, 2048:2560]),
        "w_v": np.ascontiguousarray(w_qkv[:, 2560:3072]),
        "w_o": w_out, "w_g": w_gate, "w_u": w_up, "w_d": w_down,
        "w_n1": w_norm1, "w_n2": w_norm2,
        "permM": perm,
        "onesC": np.ones((P, 1), dtype=f32),
    }


def run(inputs, trace=False):
    if "nc" not in _prog_cache:
        _prog_cache["nc"] = _build_program()
    nc = _prog_cache["nc"]
    from concourse.bass_utils import run_bass_kernel_spmd

    in_maps = [
        _host_prep(c, inputs["x"], inputs["w_norm1"], inputs["w_qkv"],
                   inputs["w_out"], inputs["w_norm2"], inputs["w_gate"],
                   inputs["w_up"], inputs["w_down"])
        for c in range(N_CORES)
    ]
    res = run_bass_kernel_spmd(nc, in_maps, core_ids=list(range(N_CORES)),
                               trace=trace)

    out = np.empty((B, S, D_MODEL), dtype=np.float32)
    for c in range(N_CORES):
        oT = res.results[c]["outT"]  # [D, 512]
        if c <= 3:
            b_small, ch_small = 0, c
            b_large, ch_large = 1, 7 - c
        else:
            b_small, ch_small = 1, 7 - c
            b_large, ch_large = 0, c
        out[b_small, ch_small * CHUNK:(ch_small + 1) * CHUNK] = oT[:, 0:256].T
        out[b_large, ch_large * CHUNK:(ch_large + 1) * CHUNK] = oT[:, 256:512].T
    return out, res


def kernel(**inputs):
    out, _ = run(inputs, trace=False)
    return out



# revision 14
# speedup vs baseline: 1.2640x; 1.2640x over previous
"""Llama MHA layer on 8 TRN2 NeuronCores.

Sharding: causal-balanced sequence sharding, no collectives. Core c owns
batch-0 chunk c and batch-1 chunk 7-c (256 tokens each). Each core
recomputes K/V projections for its chunks' prefixes locally. Per-core
KV token columns are laid out [own | prefix | zero-pad] in two fixed-size
regions (1024 / 2048 cols) so the SPMD program is identical on all cores;
padding key-tiles are neutralized by a data-driven gate column fused into
the attention V matmul (which also computes the softmax denominator).

v2: weights/activations in bf16 (halves HBM traffic so weight streams
never starve the PE), rms-norm folded into the activations via a fused
scalar_tensor_tensor (plain cos/sin tables, no per-block scaling), gpsimd
partition_broadcast instead of DRAM bounces, attention head-pairing so
score/ctx matmuls run at free-dim 512, and block-pipelined emission so
the PE queue never sits behind an unmet dependency.
"""

import numpy as np
import ml_dtypes

BF16 = ml_dtypes.bfloat16

D_MODEL = 2048
N_HEADS = 32
N_KV = 8
HEAD_DIM = 64
D_FF = 8192
ROPE_BASE = 10000.0
EPS = 1e-5
B, S = 2, 2048
CHUNK = 256
P = 128
N_CORES = 8
R_SMALL = 1024   # cols in small-chunk region
R_LARGE = 2048   # cols in large-chunk region
N_KVCOL = R_SMALL + R_LARGE   # 3072
NEG = -1e30
BLK = 512
NBLK = N_KVCOL // BLK  # 6
KD = D_MODEL // P      # 16

_prog_cache = {}


def _build_program():
    import concourse.bacc as bacc
    import concourse.bass as bass
    import concourse.mybir as mybir
    import concourse.tile as tile
    from concourse import library_config
    from concourse.alu_op_type import AluOpType

    F32 = mybir.dt.float32
    F32R = mybir.dt.float32r
    BF = mybir.dt.bfloat16
    AF = mybir.ActivationFunctionType

    nc = bacc.Bacc(None, target_bir_lowering=False)

    # ---- inputs -------------------------------------------------------
    xT = nc.dram_tensor("xT", [D_MODEL, N_KVCOL], BF, kind="ExternalInput")
    cosT = nc.dram_tensor("cosT", [P, N_KVCOL], BF, kind="ExternalInput")
    sinT = nc.dram_tensor("sinT", [P, N_KVCOL], BF, kind="ExternalInput")
    maskd = nc.dram_tensor("maskd", [2, P, 512], F32, kind="ExternalInput")
    vgate = nc.dram_tensor("vgate", [2, 16, P], BF, kind="ExternalInput")
    w_q = nc.dram_tensor("w_q", [D_MODEL, 2048], BF, kind="ExternalInput")
    w_k = nc.dram_tensor("w_k", [D_MODEL, 512], BF, kind="ExternalInput")
    w_v = nc.dram_tensor("w_v", [D_MODEL, 512], BF, kind="ExternalInput")
    w_o = nc.dram_tensor("w_o", [D_MODEL, D_MODEL], BF, kind="ExternalInput")
    w_g = nc.dram_tensor("w_g", [D_MODEL, D_FF], BF, kind="ExternalInput")
    w_u = nc.dram_tensor("w_u", [D_MODEL, D_FF], BF, kind="ExternalInput")
    w_d = nc.dram_tensor("w_d", [D_FF, D_MODEL], BF, kind="ExternalInput")
    w_n1 = nc.dram_tensor("w_n1", [D_MODEL], F32, kind="ExternalInput")
    w_n2 = nc.dram_tensor("w_n2", [D_MODEL], F32, kind="ExternalInput")
    permM = nc.dram_tensor("permM", [P, P], F32R, kind="ExternalInput")
    onesC = nc.dram_tensor("onesC", [P, 1], F32R, kind="ExternalInput")
    onesR = nc.dram_tensor("onesR", [1, P], F32, kind="ExternalInput")
    outT = nc.dram_tensor("outT", [D_MODEL, 512], F32, kind="ExternalOutput")

    INV_D = 1.0 / D_MODEL
    ATT_SCALE = 1.0 / np.sqrt(HEAD_DIM)

    _name_ctr = [0]

    def _nm(tag):
        _name_ctr[0] += 1
        return f"{tag}_{_name_ctr[0]}"

    with tile.TileContext(nc) as tc:
        import contextlib
        stack = contextlib.ExitStack()
        with stack:
            dr = stack.enter_context(tc.tile_pool(name="dr", bufs=1, space="DRAM"))
            const = stack.enter_context(tc.tile_pool(name="const", bufs=1))

            QT_s = dr.tile([2048, 512], BF, tag="QT_s", name=_nm("QT_s"))
            KT_s = dr.tile([512, N_KVCOL], BF, tag="KT_s", name=_nm("KT_s"))
            V_s = dr.tile([N_KVCOL, 512], BF, tag="V_s", name=_nm("V_s"))

            ones_sb = const.tile([P, 1], F32R, tag="ones", name=_nm("ones"))
            nc.sync.dma_start(out=ones_sb, in_=onesC.ap())
            perm_sb = const.tile([P, P], F32R, tag="perm", name=_nm("perm"))
            nc.sync.dma_start(out=perm_sb, in_=permM.ap())
            w1_sb = const.tile([P, KD], F32, tag="w1", name=_nm("w1"))
            nc.sync.dma_start(out=w1_sb, in_=w_n1.ap().rearrange("(k p) -> p k", p=P))
            w2_sb = const.tile([P, KD], F32, tag="w2", name=_nm("w2"))
            nc.sync.dma_start(out=w2_sb, in_=w_n2.ap().rearrange("(k p) -> p k", p=P))
            mask_sb = const.tile([P, 2, 512], F32, tag="mask", name=_nm("mask"))
            nc.sync.dma_start(out=mask_sb, in_=maskd.ap().rearrange("t p n -> p t n"))
            eps_sb = const.tile([P, 1], F32, tag="eps", name=_nm("eps"))
            nc.vector.memset(eps_sb, EPS)
            onesR_sb = const.tile([1, P], F32, tag="onesR", name=_nm("onesR"))
            nc.sync.dma_start(out=onesR_sb, in_=onesR.ap())

            # ========== PHASE A: rmsnorm1 + QKV proj + rope ==========
            with contextlib.ExitStack() as pa:
                tabs_p = pa.enter_context(tc.tile_pool(name="tabsA", bufs=1))
                wres_p = pa.enter_context(tc.tile_pool(name="wresA", bufs=1))
                xr_p = pa.enter_context(tc.tile_pool(name="xrA", bufs=28))
                sq_p = pa.enter_context(tc.tile_pool(name="sqA", bufs=4))
                xw_p = pa.enter_context(tc.tile_pool(name="xwA", bufs=2))
                st_p = pa.enter_context(tc.tile_pool(name="stA", bufs=4))
                ibc_p = pa.enter_context(
                    tc.tile_pool(name="ibcA", bufs=2, space="PSUM"))
                wq_p = pa.enter_context(tc.tile_pool(name="wqA", bufs=16))
                rop_p = pa.enter_context(tc.tile_pool(name="ropA", bufs=5))
                vt_p = pa.enter_context(tc.tile_pool(name="vtA", bufs=4))
                ps_ss = pa.enter_context(
                    tc.tile_pool(name="psssA", bufs=1, space="PSUM"))
                ps_mm = pa.enter_context(
                    tc.tile_pool(name="psmmA", bufs=3, space="PSUM"))
                ps_rp = pa.enter_context(
                    tc.tile_pool(name="psrpA", bufs=2, space="PSUM"))

                cos_sb = tabs_p.tile([P, N_KVCOL], BF, tag="cos", name=_nm("cos"))
                nc.sync.dma_start(out=cos_sb, in_=cosT.ap())
                sin_sb = tabs_p.tile([P, N_KVCOL], BF, tag="sin", name=_nm("sin"))
                nc.sync.dma_start(out=sin_sb, in_=sinT.ap())
                wk_sb = wres_p.tile([P, KD, 512], BF, tag="wk", name=_nm("wk"))
                nc.sync.dma_start(
                    out=wk_sb, in_=w_k.ap().rearrange("(k p) n -> p k n", p=P))
                wv_sb = wres_p.tile([P, KD, 512], BF, tag="wv", name=_nm("wv"))
                nc.sync.dma_start(
                    out=wv_sb, in_=w_v.ap().rearrange("(k p) n -> p k n", p=P))

                xws = [None, None]
                xrs_blk = {}

                def stage_S(b):
                    c0 = b * BLK
                    xrs = []
                    ssum = ps_ss.tile([1, BLK], F32, tag="ss", name=_nm("ss"))
                    for k in range(KD):
                        xr = xr_p.tile([P, BLK], BF, tag="xr", name=_nm("xr"))
                        nc.sync.dma_start(
                            out=xr, in_=xT.ap()[k * P:(k + 1) * P, c0:c0 + BLK])
                        xrs.append(xr)
                        sq = sq_p.tile([P, BLK], F32R, tag="sq", name=_nm("sq"))
                        if k % 2 == 0:
                            nc.scalar.activation(sq[:], xr[:], AF.Square)
                        else:
                            nc.vector.tensor_mul(sq[:], xr[:], xr[:])
                        nc.tensor.matmul(
                            ssum[:], ones_sb[:], sq[:],
                            start=(k == 0), stop=(k == KD - 1))
                    std = st_p.tile([1, BLK], F32, tag="std", name=_nm("std"))
                    nc.scalar.activation(
                        std[:], ssum[:], AF.Sqrt, bias=eps_sb[0:1, :], scale=INV_D)
                    inv = st_p.tile([1, BLK], F32, tag="inv", name=_nm("inv"))
                    nc.vector.reciprocal(inv[:], std[:])
                    ibc = ibc_p.tile([P, BLK], F32, tag="ibc", name=_nm("ibc"))
                    nc.tensor.matmul(ibc[:], onesR_sb[:], inv[:],
                                     start=True, stop=True)
                    xw = xw_p.tile([P, KD, BLK], BF, tag="xw", name=_nm("xw"))
                    for k in range(KD):
                        nc.vector.scalar_tensor_tensor(
                            xw[:, k, :], xrs[k][:], w1_sb[:, k:k + 1], ibc[:],
                            AluOpType.mult, AluOpType.mult)
                    xws[b % 2] = xw
                    xrs_blk[b] = xrs

                def rope_spill(psum, w, cos_ap, sin_ap, dst_ap):
                    raw = rop_p.tile([P, 512], F32R, tag="rraw", name=_nm("rraw"))[:, :w]
                    nc.scalar.activation(raw, psum, AF.Copy)
                    rot = ps_rp.tile([P, 512], F32, tag="rot", name=_nm("rot"))[:, :w]
                    nc.tensor.matmul(rot, perm_sb[:], raw, start=True, stop=True)
                    t1 = rop_p.tile([P, 512], F32, tag="t1", name=_nm("t1"))[:, :w]
                    nc.vector.tensor_mul(t1, raw, cos_ap)
                    t2 = rop_p.tile([P, 512], F32, tag="t2", name=_nm("t2"))[:, :w]
                    nc.vector.tensor_mul(t2, rot, sin_ap)
                    fin = rop_p.tile([P, 512], BF, tag="fin", name=_nm("fin"))[:, :w]
                    nc.vector.tensor_add(fin, t1, t2)
                    nc.sync.dma_start(out=dst_ap, in_=fin)

                def stage_P(b):
                    c0 = b * BLK
                    xw = xws[b % 2]
                    # K projection (+rope) -> KT_s[512, c0:c0+512]
                    for m in range(4):
                        kps = ps_mm.tile([P, 512], F32, tag="mm", name=_nm("mm"))
                        for k in range(KD):
                            nc.tensor.matmul(
                                kps[:], wk_sb[:, k, m * P:(m + 1) * P],
                                xw[:, k, :],
                                start=(k == 0), stop=(k == KD - 1))
                        rope_spill(kps[:], 512,
                                   cos_sb[:, c0:c0 + 512], sin_sb[:, c0:c0 + 512],
                                   KT_s[m * P:(m + 1) * P, c0:c0 + 512])
                    # V projection -> V_s[c0:c0+512, :]
                    for tm in range(4):
                        vps = ps_mm.tile([P, 512], F32, tag="mm", name=_nm("mm"))
                        for k in range(KD):
                            nc.tensor.matmul(
                                vps[:], xw[:, k, tm * P:(tm + 1) * P],
                                wv_sb[:, k, :],
                                start=(k == 0), stop=(k == KD - 1))
                        vt = vt_p.tile([P, 512], BF, tag="vt", name=_nm("vt"))
                        nc.scalar.activation(vt[:], vps[:], AF.Copy)
                        nc.sync.dma_start(
                            out=V_s[c0 + tm * P:c0 + (tm + 1) * P, :], in_=vt)
                    # Q projection (blocks 0 and 2 hold own columns 0:256)
                    if b in (0, 2):
                        q0 = 0 if b == 0 else 256
                        for mg in range(8):
                            qps = [ps_mm.tile([P, 512], F32, tag="mm",
                                              name=_nm("mm"))[:, :256]
                                   for _ in range(2)]
                            for k in range(KD):
                                wq_t = wq_p.tile([P, 256], BF, tag="wq",
                                                 name=_nm("wq"))
                                nc.sync.dma_start(
                                    out=wq_t,
                                    in_=w_q.ap()[k * P:(k + 1) * P,
                                                 mg * 256:(mg + 1) * 256])
                                for mi in range(2):
                                    nc.tensor.matmul(
                                        qps[mi],
                                        wq_t[:, mi * P:(mi + 1) * P],
                                        xw[:, k, 0:256],
                                        start=(k == 0), stop=(k == KD - 1))
                            for mi in range(2):
                                m = mg * 2 + mi
                                rope_spill(
                                    qps[mi], 256,
                                    cos_sb[:, c0:c0 + 256], sin_sb[:, c0:c0 + 256],
                                    QT_s[m * P:(m + 1) * P, q0:q0 + 256])
                    del xrs_blk[b]

                # pipelined emission: norm-stage one block ahead of proj-stage
                stage_S(0)
                stage_S(1)
                for b in range(NBLK):
                    stage_P(b)
                    if b + 2 < NBLK:
                        stage_S(b + 2)

            # ========== PHASE B: attention ==========
            res_p = stack.enter_context(tc.tile_pool(name="res", bufs=1))
            h2_p = stack.enter_context(tc.tile_pool(name="h2", bufs=1))
            yT = res_p.tile([P, KD, 512], F32, tag="yT", name=_nm("yT"))
            h2 = h2_p.tile([P, KD, 512], BF, tag="h2", name=_nm("h2"))
            pbc = contextlib.ExitStack()
            ctx_p = pbc.enter_context(tc.tile_pool(name="ctx", bufs=1))
            ctxt = ctx_p.tile([P, KD, 512], BF, tag="ctxt", name=_nm("ctxt"))
            with contextlib.ExitStack() as pb:
                kv_p = pb.enter_context(tc.tile_pool(name="kvB", bufs=2))
                va_p = pb.enter_context(tc.tile_pool(name="vaB", bufs=48))
                qh_p = pb.enter_context(tc.tile_pool(name="qhB", bufs=6))
                ex_p = pb.enter_context(tc.tile_pool(name="exB", bufs=6))
                sm_p = pb.enter_context(tc.tile_pool(name="smB", bufs=8))
                ps_sc = pb.enter_context(
                    tc.tile_pool(name="pssc", bufs=3, space="PSUM"))
                ps_cx = pb.enter_context(
                    tc.tile_pool(name="pscx", bufs=3, space="PSUM"))
                ps_bc = pb.enter_context(
                    tc.tile_pool(name="psbc", bufs=1, space="PSUM"))
                ps_jk = pb.enter_context(
                    tc.tile_pool(name="psjk", bufs=1, space="PSUM"))

                for g in range(N_KV):
                    ksb = kv_p.tile([64, N_KVCOL], BF, tag="ksb", name=_nm("ksb"))
                    nc.sync.dma_start(
                        out=ksb, in_=KT_s[g * 64:(g + 1) * 64, :])
                    for cc in range(2):
                        nkt = 8 if cc == 0 else 16
                        kc0 = 0 if cc == 0 else R_SMALL
                        vaugs = []
                        for kt in range(nkt):
                            va = va_p.tile([P, 65], BF, tag="va", name=_nm("va"))
                            nc.sync.dma_start(
                                out=va[:, 0:64],
                                in_=V_s[kc0 + kt * P:kc0 + (kt + 1) * P,
                                        g * 64:(g + 1) * 64])
                            nc.sync.dma_start(
                                out=va[:, 64:65],
                                in_=vgate.ap()[cc, kt, :].rearrange(
                                    "(p o) -> p o", o=1))
                            vaugs.append(va)
                        for hp in range(2):
                            h0 = g * 4 + 2 * hp
                            qh2 = qh_p.tile([64, 512], BF, tag="qh", name=_nm("qh"))
                            nc.sync.dma_start(
                                out=qh2[:, 0:256],
                                in_=QT_s[h0 * 64:(h0 + 1) * 64,
                                         cc * 256:(cc + 1) * 256])
                            nc.sync.dma_start(
                                out=qh2[:, 256:512],
                                in_=QT_s[(h0 + 1) * 64:(h0 + 2) * 64,
                                         cc * 256:(cc + 1) * 256])
                            cxp = ps_cx.tile([65, 512], F32, tag="cx", name=_nm("cx"))

                            exs = {}

                            def emit_sc(kt):
                                scp = ps_sc.tile([P, 512], F32, tag="sc",
                                                 name=_nm("sc"))
                                nc.tensor.matmul(
                                    scp[:],
                                    ksb[:, kc0 + kt * P:kc0 + (kt + 1) * P],
                                    qh2[:], start=True, stop=True)
                                if kt < 2:
                                    nc.vector.tensor_add(
                                        scp[:], scp[:], mask_sb[:, kt, :])
                                ex = ex_p.tile([P, 512], BF, tag="ex", name=_nm("ex"))
                                nc.scalar.activation(
                                    ex[:], scp[:], AF.Exp, scale=ATT_SCALE)
                                exs[kt] = ex

                            # scores run 2 key-tiles ahead of the ctx matmuls;
                            # a junk matmul per iteration keeps PE duty above
                            # the HAM re-throttle threshold while exp (ACT) is
                            # the true critical path
                            emit_sc(0)
                            if nkt > 1:
                                emit_sc(1)
                            for kt in range(nkt):
                                if kt + 2 < nkt:
                                    emit_sc(kt + 2)
                                jk = ps_jk.tile([P, 512], F32, tag="jk",
                                                name=_nm("jk"))
                                nc.tensor.matmul(
                                    jk[:], ksb[:, kc0:kc0 + P], qh2[:],
                                    start=True, stop=True)
                                nc.tensor.matmul(
                                    cxp[:], vaugs[kt][:], exs.pop(kt),
                                    start=(kt == 0), stop=(kt == nkt - 1))
                            rec = sm_p.tile([1, 512], F32, tag="rec", name=_nm("rec"))
                            nc.vector.reciprocal(rec[:], cxp[64:65, :])
                            bcr = ps_bc.tile([64, 512], F32, tag="bcr", name=_nm("bcr"))
                            nc.tensor.matmul(bcr[:], onesR_sb[:, 0:64], rec[:],
                                             start=True, stop=True)
                            bcs = sm_p.tile([64, 512], F32, tag="bcs", name=_nm("bcs"))
                            nc.vector.tensor_copy(bcs[:], bcr[:])
                            tmp = sm_p.tile([64, 512], BF, tag="cno", name=_nm("cno"))
                            nc.vector.tensor_mul(tmp[:], cxp[0:64, :], bcs[:])
                            a = g * 2 + hp
                            nc.sync.dma_start(
                                out=ctxt[0:64, a, cc * 256:(cc + 1) * 256],
                                in_=tmp[:, 0:256])
                            nc.sync.dma_start(
                                out=ctxt[64:128, a, cc * 256:(cc + 1) * 256],
                                in_=tmp[:, 256:512])

            # ========== PHASE C: out-proj + residual + rmsnorm2 ==========
            with contextlib.ExitStack() as pc:
                xo_p = pc.enter_context(tc.tile_pool(name="xoC", bufs=1))
                wo_p = pc.enter_context(tc.tile_pool(name="woC", bufs=24))
                tmp2_p = pc.enter_context(tc.tile_pool(name="tmpC", bufs=4))
                ibc2_p = pc.enter_context(
                    tc.tile_pool(name="ibc2C", bufs=1, space="PSUM"))
                ps_y = pc.enter_context(
                    tc.tile_pool(name="psyC", bufs=4, space="PSUM"))
                ps_s2 = pc.enter_context(
                    tc.tile_pool(name="pss2", bufs=1, space="PSUM"))

                xo = xo_p.tile([P, KD, 512], BF, tag="xo", name=_nm("xo"))
                for k in range(KD):
                    nc.sync.dma_start(
                        out=xo[:, k, 0:256], in_=xT.ap()[k * P:(k + 1) * P, 0:256])
                    nc.sync.dma_start(
                        out=xo[:, k, 256:512],
                        in_=xT.ap()[k * P:(k + 1) * P, R_SMALL:R_SMALL + 256])

                for mg in range(8):
                    yps = [ps_y.tile([P, 512], F32, tag="y", name=_nm("y"))
                           for _ in range(2)]
                    for k in range(KD):
                        wo_t = wo_p.tile([P, 256], BF, tag="wo", name=_nm("wo"))
                        nc.sync.dma_start(
                            out=wo_t,
                            in_=w_o.ap()[k * P:(k + 1) * P,
                                         mg * 256:(mg + 1) * 256])
                        for mi in range(2):
                            nc.tensor.matmul(
                                yps[mi], wo_t[:, mi * P:(mi + 1) * P],
                                ctxt[:, k, :],
                                start=(k == 0), stop=(k == KD - 1))
                    for mi in range(2):
                        m = mg * 2 + mi
                        nc.vector.tensor_add(yT[:, m, :], yps[mi][:], xo[:, m, :])

                ss2 = ps_s2.tile([1, 512], F32, tag="ss2", name=_nm("ss2"))
                for m in range(KD):
                    sq2 = tmp2_p.tile([P, 512], F32R, tag="sq2", name=_nm("sq2"))
                    nc.scalar.activation(sq2[:], yT[:, m, :], AF.Square)
                    nc.tensor.matmul(ss2[:], ones_sb[:], sq2[:],
                                     start=(m == 0), stop=(m == KD - 1))
                std2 = tmp2_p.tile([1, 512], F32, tag="std2", name=_nm("std2"))
                nc.scalar.activation(std2[:], ss2[:], AF.Sqrt,
                                     bias=eps_sb[0:1, :], scale=INV_D)
                inv2 = tmp2_p.tile([1, 512], F32, tag="inv2", name=_nm("inv2"))
                nc.vector.reciprocal(inv2[:], std2[:])
                ibc2 = ibc2_p.tile([P, 512], F32, tag="ibc2", name=_nm("ibc2"))
                nc.tensor.matmul(ibc2[:], onesR_sb[:], inv2[:],
                                 start=True, stop=True)
                for m in range(KD):
                    nc.vector.scalar_tensor_tensor(
                        h2[:, m, :], yT[:, m, :], w2_sb[:, m:m + 1], ibc2[:],
                        AluOpType.mult, AluOpType.mult)

            pbc.close()

            # ========== PHASE D: SwiGLU MLP ==========
            with contextlib.ExitStack() as pd:
                ht_p = pd.enter_context(tc.tile_pool(name="htD", bufs=32))
                y2_p = pd.enter_context(tc.tile_pool(name="y2D", bufs=1))
                wgu_p = pd.enter_context(tc.tile_pool(name="wguD", bufs=24))
                wd_p = pd.enter_context(tc.tile_pool(name="wdD", bufs=24))
                sil_p = pd.enter_context(tc.tile_pool(name="silD", bufs=4))
                o_p = pd.enter_context(tc.tile_pool(name="oD", bufs=4))
                ps_gu = pd.enter_context(
                    tc.tile_pool(name="psgu", bufs=6, space="PSUM"))
                ps_d = pd.enter_context(
                    tc.tile_pool(name="psd", bufs=2, space="PSUM"))

                y2acc = y2_p.tile([P, KD, 512], F32, tag="y2", name=_nm("y2"))
                for grp in range(4):
                    f0 = grp * 2048
                    hts = []
                    for fg in range(8):
                        gps = [ps_gu.tile([P, 512], F32, tag="gu", name=_nm("gu"))
                               for _ in range(2)]
                        ups = [ps_gu.tile([P, 512], F32, tag="gu", name=_nm("gu"))
                               for _ in range(2)]
                        for k in range(KD):
                            wg_t = wgu_p.tile([P, 256], BF, tag="wg", name=_nm("wg"))
                            nc.sync.dma_start(
                                out=wg_t,
                                in_=w_g.ap()[k * P:(k + 1) * P,
                                             f0 + fg * 256:f0 + (fg + 1) * 256])
                            wu_t = wgu_p.tile([P, 256], BF, tag="wu", name=_nm("wu"))
                            nc.sync.dma_start(
                                out=wu_t,
                                in_=w_u.ap()[k * P:(k + 1) * P,
                                             f0 + fg * 256:f0 + (fg + 1) * 256])
                            for mi in range(2):
                                nc.tensor.matmul(
                                    gps[mi], wg_t[:, mi * P:(mi + 1) * P],
                                    h2[:, k, :],
                                    start=(k == 0), stop=(k == KD - 1))
                                nc.tensor.matmul(
                                    ups[mi], wu_t[:, mi * P:(mi + 1) * P],
                                    h2[:, k, :],
                                    start=(k == 0), stop=(k == KD - 1))
                        for mi in range(2):
                            sil = sil_p.tile([P, 512], F32, tag="sil", name=_nm("sil"))
                            nc.scalar.activation(sil[:], gps[mi][:], AF.Silu)
                            ht = ht_p.tile([P, 512], BF, tag="ht", name=_nm("ht"))
                            nc.vector.tensor_mul(ht[:], sil[:], ups[mi][:])
                            hts.append(ht)
                    for mg in range(8):
                        dps = [ps_d.tile([P, 512], F32, tag="d", name=_nm("d"))
                               for _ in range(2)]
                        for kk in range(16):
                            kr = f0 + kk * P
                            wd_t = wd_p.tile([P, 256], BF, tag="wd", name=_nm("wd"))
                            nc.sync.dma_start(
                                out=wd_t,
                                in_=w_d.ap()[kr:kr + P, mg * 256:(mg + 1) * 256])
                            for mi in range(2):
                                nc.tensor.matmul(
                                    dps[mi], wd_t[:, mi * P:(mi + 1) * P],
                                    hts[kk][:],
                                    start=(kk == 0), stop=(kk == 15))
                        for mi in range(2):
                            m = mg * 2 + mi
                            if grp == 0:
                                nc.vector.tensor_copy(y2acc[:, m, :], dps[mi][:])
                            else:
                                nc.vector.tensor_add(
                                    y2acc[:, m, :], y2acc[:, m, :], dps[mi][:])

                for m in range(KD):
                    o = o_p.tile([P, 512], F32, tag="o", name=_nm("o"))
                    nc.vector.tensor_add(o[:], y2acc[:, m, :], yT[:, m, :])
                    nc.sync.dma_start(
                        out=outT.ap()[m * P:(m + 1) * P, :], in_=o)

    nc.compile()
    return nc


# ======================= host-side prep =======================

def _host_prep_const(w_qkv, w_out, w_gate, w_up, w_down, w_norm1, w_norm2):
    """Core-independent tensors (computed once, shared across cores)."""
    f32 = np.float32
    # diagonal causal masks, duplicated for the paired heads (cols 0:256 and
    # 256:512 are the same 256 queries for two different heads)
    m = np.zeros((2, P, 512), dtype=f32)
    j = np.arange(256)[None, :]
    k_ = np.arange(P)[:, None]
    m[0, :, 0:256] = np.where(k_ > j, NEG, 0.0)
    m[0, :, 256:512] = m[0, :, 0:256]
    m[1, :, 0:256] = np.where(k_ + P > j, NEG, 0.0)
    m[1, :, 256:512] = m[1, :, 0:256]

    perm = np.zeros((P, P), dtype=f32)
    for r in range(P):
        d = r % 64
        s = r + 32 if d < 32 else r - 32
        perm[s, r] = 1.0

    return {
        "maskd": m,
        "w_q": np.ascontiguousarray(w_qkv[:, :2048]).astype(BF16),
        "w_k": np.ascontiguousarray(w_qkv[:, 2048:2560]).astype(BF16),
        "w_v": np.ascontiguousarray(w_qkv[:, 2560:3072]).astype(BF16),
        "w_o": np.asarray(w_out).astype(BF16),
        "w_g": np.asarray(w_gate).astype(BF16),
        "w_u": np.asarray(w_up).astype(BF16),
        "w_d": np.asarray(w_down).astype(BF16),
        "w_n1": np.asarray(w_norm1, dtype=f32),
        "w_n2": np.asarray(w_norm2, dtype=f32),
        "permM": perm,
        "onesC": np.ones((P, 1), dtype=f32),
        "onesR": np.ones((1, P), dtype=f32),
    }


def _host_prep_core(c, x, shared):
    """Per-core layout/slicing + rope tables."""
    f32 = np.float32
    if c <= 3:
        b_small, ch_small = 0, c
        b_large, ch_large = 1, 7 - c
    else:
        b_small, ch_small = 1, 7 - c
        b_large, ch_large = 0, c

    xT_full0 = x[b_small].T  # [D, S]
    xT_full1 = x[b_large].T

    xTc = np.zeros((D_MODEL, N_KVCOL), dtype=f32)
    pos = np.zeros(N_KVCOL, dtype=np.int64)
    # small region: [own | prefix | pad]
    o0 = ch_small * CHUNK
    xTc[:, 0:CHUNK] = xT_full0[:, o0:o0 + CHUNK]
    pos[0:CHUNK] = np.arange(o0, o0 + CHUNK)
    npre = o0
    xTc[:, CHUNK:CHUNK + npre] = xT_full0[:, 0:npre]
    pos[CHUNK:CHUNK + npre] = np.arange(npre)
    # large region
    o1 = ch_large * CHUNK
    xTc[:, R_SMALL:R_SMALL + CHUNK] = xT_full1[:, o1:o1 + CHUNK]
    pos[R_SMALL:R_SMALL + CHUNK] = np.arange(o1, o1 + CHUNK)
    npre1 = o1
    xTc[:, R_SMALL + CHUNK:R_SMALL + CHUNK + npre1] = xT_full1[:, 0:npre1]
    pos[R_SMALL + CHUNK:R_SMALL + CHUNK + npre1] = np.arange(npre1)

    # rope tables, replicated for 2 heads per 128 partitions, sign folded
    inv_freq = (ROPE_BASE ** (-np.arange(0, HEAD_DIM, 2, dtype=np.float64)
                              / HEAD_DIM))  # [32]
    ang = pos[None, :] * inv_freq[:, None]          # [32, N_KVCOL]
    cos32 = np.cos(ang)
    sin32 = np.sin(ang)
    cosT = np.empty((P, N_KVCOL), dtype=f32)
    sinT = np.empty((P, N_KVCOL), dtype=f32)
    for hh in range(2):
        r = hh * 64
        cosT[r:r + 32] = cos32
        cosT[r + 32:r + 64] = cos32
        sinT[r:r + 32] = -sin32
        sinT[r + 32:r + 64] = sin32

    # gate column: 1.0 for real key-tiles, 0.0 for padding
    vgate = np.zeros((2, 16, P), dtype=f32)
    vgate[0, :2 + 2 * ch_small, :] = 1.0
    vgate[1, :2 + 2 * ch_large, :] = 1.0

    d = {
        "xT": np.ascontiguousarray(xTc).astype(BF16),
        "cosT": cosT.astype(BF16), "sinT": sinT.astype(BF16),
        "vgate": vgate.astype(BF16),
    }
    d.update(shared)
    return d


def run(inputs, trace=False):
    if "nc" not in _prog_cache:
        _prog_cache["nc"] = _build_program()
    nc = _prog_cache["nc"]
    from concourse.bass_utils import run_bass_kernel_spmd

    shared = _host_prep_const(
        np.asarray(inputs["w_qkv"]), np.asarray(inputs["w_out"]),
        np.asarray(inputs["w_gate"]), np.asarray(inputs["w_up"]),
        np.asarray(inputs["w_down"]), np.asarray(inputs["w_norm1"]),
        np.asarray(inputs["w_norm2"]))
    x = np.asarray(inputs["x"])
    in_maps = [_host_prep_core(c, x, shared) for c in range(N_CORES)]
    res = run_bass_kernel_spmd(nc, in_maps, core_ids=list(range(N_CORES)),
                               trace=trace)

    out = np.empty((B, S, D_MODEL), dtype=np.float32)
    for c in range(N_CORES):
        oT = res.results[c]["outT"]  # [D, 512]
        if c <= 3:
            b_small, ch_small = 0, c
            b_large, ch_large = 1, 7 - c
        else:
            b_small, ch_small = 1, 7 - c
            b_large, ch_large = 0, c
        out[b_small, ch_small * CHUNK:(ch_small + 1) * CHUNK] = oT[:, 0:256].T
        out[b_large, ch_large * CHUNK:(ch_large + 1) * CHUNK] = oT[:, 256:512].T
    return out, res


def kernel(**inputs):
    out, _ = run(inputs, trace=False)
    return out


# revision 16
# speedup vs baseline: 1.3930x; 1.1020x over previous
"""Llama MHA layer on 8 TRN2 NeuronCores.

Sharding: causal-balanced sequence sharding, no collectives. Core c owns
batch-0 chunk c and batch-1 chunk 7-c (256 tokens each). Each core
recomputes K/V projections for its chunks' prefixes locally. Per-core
KV token columns are laid out [own | prefix | zero-pad] in two fixed-size
regions (1024 / 2048 cols) so the SPMD program is identical on all cores;
padding key-tiles are neutralized by a data-driven gate column fused into
the attention V matmul (which also computes the softmax denominator).

v2: weights/activations in bf16 (halves HBM traffic so weight streams
never starve the PE), rms-norm folded into the activations via a fused
scalar_tensor_tensor (plain cos/sin tables, no per-block scaling), gpsimd
partition_broadcast instead of DRAM bounces, attention head-pairing so
score/ctx matmuls run at free-dim 512, and block-pipelined emission so
the PE queue never sits behind an unmet dependency.
"""

import numpy as np
import ml_dtypes

BF16 = ml_dtypes.bfloat16

D_MODEL = 2048
N_HEADS = 32
N_KV = 8
HEAD_DIM = 64
D_FF = 8192
ROPE_BASE = 10000.0
EPS = 1e-5
B, S = 2, 2048
CHUNK = 256
P = 128
N_CORES = 8
R_SMALL = 1024   # cols in small-chunk region
R_LARGE = 2048   # cols in large-chunk region
N_KVCOL = R_SMALL + R_LARGE   # 3072
NEG = -1e30
BLK = 512
NBLK = N_KVCOL // BLK  # 6
KD = D_MODEL // P      # 16

_prog_cache = {}


def _build_program():
    import concourse.bacc as bacc
    import concourse.bass as bass
    import concourse.mybir as mybir
    import concourse.tile as tile
    from concourse import library_config
    from concourse.alu_op_type import AluOpType

    F32 = mybir.dt.float32
    F32R = mybir.dt.float32r
    BF = mybir.dt.bfloat16
    AF = mybir.ActivationFunctionType

    nc = bacc.Bacc(None, target_bir_lowering=False)

    # ---- inputs -------------------------------------------------------
    xT = nc.dram_tensor("xT", [D_MODEL, N_KVCOL], BF, kind="ExternalInput")
    cosT = nc.dram_tensor("cosT", [P, N_KVCOL], BF, kind="ExternalInput")
    sinT = nc.dram_tensor("sinT", [P, N_KVCOL], BF, kind="ExternalInput")
    maskd = nc.dram_tensor("maskd", [2, P, 512], F32, kind="ExternalInput")
    vgate = nc.dram_tensor("vgate", [2, 16, P], BF, kind="ExternalInput")
    w_q = nc.dram_tensor("w_q", [D_MODEL, 2048], BF, kind="ExternalInput")
    w_k = nc.dram_tensor("w_k", [D_MODEL, 512], BF, kind="ExternalInput")
    w_v = nc.dram_tensor("w_v", [D_MODEL, 512], BF, kind="ExternalInput")
    w_o = nc.dram_tensor("w_o", [D_MODEL, D_MODEL], BF, kind="ExternalInput")
    w_g = nc.dram_tensor("w_g", [D_MODEL, D_FF], BF, kind="ExternalInput")
    w_u = nc.dram_tensor("w_u", [D_MODEL, D_FF], BF, kind="ExternalInput")
    w_d = nc.dram_tensor("w_d", [D_FF, D_MODEL], BF, kind="ExternalInput")
    w_n1 = nc.dram_tensor("w_n1", [D_MODEL], F32, kind="ExternalInput")
    w_n2 = nc.dram_tensor("w_n2", [D_MODEL], F32, kind="ExternalInput")
    permM = nc.dram_tensor("permM", [P, P], F32R, kind="ExternalInput")
    onesC = nc.dram_tensor("onesC", [P, 1], F32R, kind="ExternalInput")
    onesR = nc.dram_tensor("onesR", [1, P], F32, kind="ExternalInput")
    outT = nc.dram_tensor("outT", [D_MODEL, 512], F32, kind="ExternalOutput")

    INV_D = 1.0 / D_MODEL
    ATT_SCALE = 1.0 / np.sqrt(HEAD_DIM)

    _name_ctr = [0]

    def _nm(tag):
        _name_ctr[0] += 1
        return f"{tag}_{_name_ctr[0]}"

    with tile.TileContext(nc) as tc:
        import contextlib
        stack = contextlib.ExitStack()
        with stack:
            dr = stack.enter_context(tc.tile_pool(name="dr", bufs=1, space="DRAM"))
            const = stack.enter_context(tc.tile_pool(name="const", bufs=1))

            QT_s = dr.tile([2048, 512], BF, tag="QT_s", name=_nm("QT_s"))
            KT_s = dr.tile([512, N_KVCOL], BF, tag="KT_s", name=_nm("KT_s"))
            V_s = dr.tile([N_KVCOL, 512], BF, tag="V_s", name=_nm("V_s"))

            ones_sb = const.tile([P, 1], F32R, tag="ones", name=_nm("ones"))
            nc.sync.dma_start(out=ones_sb, in_=onesC.ap())
            perm_sb = const.tile([P, P], F32R, tag="perm", name=_nm("perm"))
            nc.sync.dma_start(out=perm_sb, in_=permM.ap())
            w1_sb = const.tile([P, KD], F32, tag="w1", name=_nm("w1"))
            nc.sync.dma_start(out=w1_sb, in_=w_n1.ap().rearrange("(k p) -> p k", p=P))
            w2_sb = const.tile([P, KD], F32, tag="w2", name=_nm("w2"))
            nc.sync.dma_start(out=w2_sb, in_=w_n2.ap().rearrange("(k p) -> p k", p=P))
            mask_sb = const.tile([P, 2, 512], F32, tag="mask", name=_nm("mask"))
            nc.sync.dma_start(out=mask_sb, in_=maskd.ap().rearrange("t p n -> p t n"))
            eps_sb = const.tile([P, 1], F32, tag="eps", name=_nm("eps"))
            nc.vector.memset(eps_sb, EPS)
            onesR_sb = const.tile([1, P], F32, tag="onesR", name=_nm("onesR"))
            nc.sync.dma_start(out=onesR_sb, in_=onesR.ap())

            # ========== PHASE A: rmsnorm1 + QKV proj + rope ==========
            with contextlib.ExitStack() as pa:
                tabs_p = pa.enter_context(tc.tile_pool(name="tabsA", bufs=1))
                wres_p = pa.enter_context(tc.tile_pool(name="wresA", bufs=1))
                xr_p = pa.enter_context(tc.tile_pool(name="xrA", bufs=28))
                sq_p = pa.enter_context(tc.tile_pool(name="sqA", bufs=4))
                xw_p = pa.enter_context(tc.tile_pool(name="xwA", bufs=2))
                st_p = pa.enter_context(tc.tile_pool(name="stA", bufs=4))
                ibc_p = pa.enter_context(
                    tc.tile_pool(name="ibcA", bufs=2, space="PSUM"))
                wq_p = pa.enter_context(tc.tile_pool(name="wqA", bufs=16))
                rop_p = pa.enter_context(tc.tile_pool(name="ropA", bufs=5))
                vt_p = pa.enter_context(tc.tile_pool(name="vtA", bufs=4))
                ps_ss = pa.enter_context(
                    tc.tile_pool(name="psssA", bufs=1, space="PSUM"))
                ps_mm = pa.enter_context(
                    tc.tile_pool(name="psmmA", bufs=3, space="PSUM"))
                ps_rp = pa.enter_context(
                    tc.tile_pool(name="psrpA", bufs=2, space="PSUM"))

                cos_sb = tabs_p.tile([P, N_KVCOL], BF, tag="cos", name=_nm("cos"))
                nc.sync.dma_start(out=cos_sb, in_=cosT.ap())
                sin_sb = tabs_p.tile([P, N_KVCOL], BF, tag="sin", name=_nm("sin"))
                nc.sync.dma_start(out=sin_sb, in_=sinT.ap())
                wk_sb = wres_p.tile([P, KD, 512], BF, tag="wk", name=_nm("wk"))
                nc.sync.dma_start(
                    out=wk_sb, in_=w_k.ap().rearrange("(k p) n -> p k n", p=P))
                wv_sb = wres_p.tile([P, KD, 512], BF, tag="wv", name=_nm("wv"))
                nc.sync.dma_start(
                    out=wv_sb, in_=w_v.ap().rearrange("(k p) n -> p k n", p=P))

                xws = [None, None]
                xrs_blk = {}

                def stage_S(b):
                    c0 = b * BLK
                    xrs = []
                    ssum = ps_ss.tile([1, BLK], F32, tag="ss", name=_nm("ss"))
                    for k in range(KD):
                        xr = xr_p.tile([P, BLK], BF, tag="xr", name=_nm("xr"))
                        nc.sync.dma_start(
                            out=xr, in_=xT.ap()[k * P:(k + 1) * P, c0:c0 + BLK])
                        xrs.append(xr)
                        sq = sq_p.tile([P, BLK], F32R, tag="sq", name=_nm("sq"))
                        if k % 2 == 0:
                            nc.scalar.activation(sq[:], xr[:], AF.Square)
                        else:
                            nc.vector.tensor_mul(sq[:], xr[:], xr[:])
                        nc.tensor.matmul(
                            ssum[:], ones_sb[:], sq[:],
                            start=(k == 0), stop=(k == KD - 1))
                    std = st_p.tile([1, BLK], F32, tag="std", name=_nm("std"))
                    nc.scalar.activation(
                        std[:], ssum[:], AF.Sqrt, bias=eps_sb[0:1, :], scale=INV_D)
                    inv = st_p.tile([1, BLK], F32, tag="inv", name=_nm("inv"))
                    nc.vector.reciprocal(inv[:], std[:])
                    ibc = ibc_p.tile([P, BLK], F32, tag="ibc", name=_nm("ibc"))
                    nc.tensor.matmul(ibc[:], onesR_sb[:], inv[:],
                                     start=True, stop=True)
                    xw = xw_p.tile([P, KD, BLK], BF, tag="xw", name=_nm("xw"))
                    for k in range(KD):
                        nc.vector.scalar_tensor_tensor(
                            xw[:, k, :], xrs[k][:], w1_sb[:, k:k + 1], ibc[:],
                            AluOpType.mult, AluOpType.mult)
                    xws[b % 2] = xw
                    xrs_blk[b] = xrs

                def rope_spill(psum, w, cos_ap, sin_ap, dst_ap):
                    raw = rop_p.tile([P, 512], F32R, tag="rraw", name=_nm("rraw"))[:, :w]
                    nc.scalar.activation(raw, psum, AF.Copy)
                    rot = ps_rp.tile([P, 512], F32, tag="rot", name=_nm("rot"))[:, :w]
                    nc.tensor.matmul(rot, perm_sb[:], raw, start=True, stop=True)
                    t1 = rop_p.tile([P, 512], F32, tag="t1", name=_nm("t1"))[:, :w]
                    nc.vector.tensor_mul(t1, raw, cos_ap)
                    t2 = rop_p.tile([P, 512], F32, tag="t2", name=_nm("t2"))[:, :w]
                    nc.vector.tensor_mul(t2, rot, sin_ap)
                    fin = rop_p.tile([P, 512], BF, tag="fin", name=_nm("fin"))[:, :w]
                    nc.vector.tensor_add(fin, t1, t2)
                    nc.sync.dma_start(out=dst_ap, in_=fin)

                def stage_P(b):
                    c0 = b * BLK
                    xw = xws[b % 2]
                    # K projection (+rope) -> KT_s[512, c0:c0+512]
                    for m in range(4):
                        kps = ps_mm.tile([P, 512], F32, tag="mm", name=_nm("mm"))
                        for k in range(KD):
                            nc.tensor.matmul(
                                kps[:], wk_sb[:, k, m * P:(m + 1) * P],
                                xw[:, k, :],
                                start=(k == 0), stop=(k == KD - 1))
                        rope_spill(kps[:], 512,
                                   cos_sb[:, c0:c0 + 512], sin_sb[:, c0:c0 + 512],
                                   KT_s[m * P:(m + 1) * P, c0:c0 + 512])
                    # V projection -> V_s[c0:c0+512, :]
                    for tm in range(4):
                        vps = ps_mm.tile([P, 512], F32, tag="mm", name=_nm("mm"))
                        for k in range(KD):
                            nc.tensor.matmul(
                                vps[:], xw[:, k, tm * P:(tm + 1) * P],
                                wv_sb[:, k, :],
                                start=(k == 0), stop=(k == KD - 1))
                        vt = vt_p.tile([P, 512], BF, tag="vt", name=_nm("vt"))
                        nc.scalar.activation(vt[:], vps[:], AF.Copy)
                        nc.sync.dma_start(
                            out=V_s[c0 + tm * P:c0 + (tm + 1) * P, :], in_=vt)
                    # Q projection (blocks 0 and 2 hold own columns 0:256)
                    if b in (0, 2):
                        q0 = 0 if b == 0 else 256
                        for mg in range(8):
                            qps = [ps_mm.tile([P, 512], F32, tag="mm",
                                              name=_nm("mm"))[:, :256]
                                   for _ in range(2)]
                            for k in range(KD):
                                wq_t = wq_p.tile([P, 256], BF, tag="wq",
                                                 name=_nm("wq"))
                                nc.sync.dma_start(
                                    out=wq_t,
                                    in_=w_q.ap()[k * P:(k + 1) * P,
                                                 mg * 256:(mg + 1) * 256])
                                for mi in range(2):
                                    nc.tensor.matmul(
                                        qps[mi],
                                        wq_t[:, mi * P:(mi + 1) * P],
                                        xw[:, k, 0:256],
                                        start=(k == 0), stop=(k == KD - 1))
                            for mi in range(2):
                                m = mg * 2 + mi
                                rope_spill(
                                    qps[mi], 256,
                                    cos_sb[:, c0:c0 + 256], sin_sb[:, c0:c0 + 256],
                                    QT_s[m * P:(m + 1) * P, q0:q0 + 256])
                    del xrs_blk[b]

                # pipelined emission: norm-stage one block ahead of proj-stage
                stage_S(0)
                stage_S(1)
                for b in range(NBLK):
                    stage_P(b)
                    if b + 2 < NBLK:
                        stage_S(b + 2)

            # ========== PHASE B: attention ==========
            res_p = stack.enter_context(tc.tile_pool(name="res", bufs=1))
            h2_p = stack.enter_context(tc.tile_pool(name="h2", bufs=1))
            yT = res_p.tile([P, KD, 512], F32, tag="yT", name=_nm("yT"))
            h2 = h2_p.tile([P, KD, 512], BF, tag="h2", name=_nm("h2"))
            pbc = contextlib.ExitStack()
            ctx_p = pbc.enter_context(tc.tile_pool(name="ctx", bufs=1))
            ctxt = ctx_p.tile([P, KD, 512], BF, tag="ctxt", name=_nm("ctxt"))
            with contextlib.ExitStack() as pb:
                kv_p = pb.enter_context(tc.tile_pool(name="kvB", bufs=2))
                va_p = pb.enter_context(tc.tile_pool(name="vaB", bufs=48))
                qh_p = pb.enter_context(tc.tile_pool(name="qhB", bufs=6))
                ex_p = pb.enter_context(tc.tile_pool(name="exB", bufs=6))
                sm_p = pb.enter_context(tc.tile_pool(name="smB", bufs=8))
                ps_sc = pb.enter_context(
                    tc.tile_pool(name="pssc", bufs=3, space="PSUM"))
                ps_cx = pb.enter_context(
                    tc.tile_pool(name="pscx", bufs=3, space="PSUM"))
                ps_bc = pb.enter_context(
                    tc.tile_pool(name="psbc", bufs=1, space="PSUM"))
                ps_jk = pb.enter_context(
                    tc.tile_pool(name="psjk", bufs=1, space="PSUM"))

                for g in range(N_KV):
                    ksb = kv_p.tile([P, N_KVCOL], BF, tag="ksb", name=_nm("ksb"))
                    nc.vector.memset(ksb[64:128, :], 0.0)
                    nc.sync.dma_start(
                        out=ksb[0:64, :], in_=KT_s[g * 64:(g + 1) * 64, :])
                    for cc in range(2):
                        nkt = 8 if cc == 0 else 16
                        kc0 = 0 if cc == 0 else R_SMALL
                        vaugs = []
                        for kt in range(nkt):
                            va = va_p.tile([P, 65], BF, tag="va", name=_nm("va"))
                            nc.sync.dma_start(
                                out=va[:, 0:64],
                                in_=V_s[kc0 + kt * P:kc0 + (kt + 1) * P,
                                        g * 64:(g + 1) * 64])
                            nc.sync.dma_start(
                                out=va[:, 64:65],
                                in_=vgate.ap()[cc, kt, :].rearrange(
                                    "(p o) -> p o", o=1))
                            vaugs.append(va)
                        for hp in range(2):
                            h0 = g * 4 + 2 * hp
                            qh2 = qh_p.tile([P, 512], BF, tag="qh", name=_nm("qh"))
                            nc.vector.memset(qh2[64:128, :], 0.0)
                            nc.sync.dma_start(
                                out=qh2[0:64, 0:256],
                                in_=QT_s[h0 * 64:(h0 + 1) * 64,
                                         cc * 256:(cc + 1) * 256])
                            nc.sync.dma_start(
                                out=qh2[0:64, 256:512],
                                in_=QT_s[(h0 + 1) * 64:(h0 + 2) * 64,
                                         cc * 256:(cc + 1) * 256])
                            cxp = ps_cx.tile([65, 512], F32, tag="cx", name=_nm("cx"))

                            exs = {}

                            def emit_sc(kt):
                                scp = ps_sc.tile([P, 512], F32, tag="sc",
                                                 name=_nm("sc"))
                                nc.tensor.matmul(
                                    scp[:],
                                    ksb[:, kc0 + kt * P:kc0 + (kt + 1) * P],
                                    qh2[:], start=True, stop=True)
                                if kt < 2:
                                    nc.vector.tensor_add(
                                        scp[:], scp[:], mask_sb[:, kt, :])
                                ex = ex_p.tile([P, 512], BF, tag="ex", name=_nm("ex"))
                                nc.scalar.activation(
                                    ex[:], scp[:], AF.Exp, scale=ATT_SCALE)
                                exs[kt] = ex

                            # scores run 2 key-tiles ahead of the ctx matmuls;
                            # a junk matmul per iteration keeps PE duty above
                            # the HAM re-throttle threshold while exp (ACT) is
                            # the true critical path
                            emit_sc(0)
                            if nkt > 1:
                                emit_sc(1)
                            for kt in range(nkt):
                                if kt + 2 < nkt:
                                    emit_sc(kt + 2)
                                jk = ps_jk.tile([P, 512], F32, tag="jk",
                                                name=_nm("jk"))
                                nc.tensor.matmul(
                                    jk[:], ksb[:, kc0:kc0 + P], qh2[:],
                                    start=True, stop=True)
                                nc.tensor.matmul(
                                    cxp[:], vaugs[kt][:], exs.pop(kt),
                                    start=(kt == 0), stop=(kt == nkt - 1))
                            rec = sm_p.tile([1, 512], F32, tag="rec", name=_nm("rec"))
                            nc.vector.reciprocal(rec[:], cxp[64:65, :])
                            bcr = ps_bc.tile([64, 512], F32, tag="bcr", name=_nm("bcr"))
                            nc.tensor.matmul(bcr[:], onesR_sb[:, 0:64], rec[:],
                                             start=True, stop=True)
                            bcs = sm_p.tile([64, 512], F32, tag="bcs", name=_nm("bcs"))
                            nc.vector.tensor_copy(bcs[:], bcr[:])
                            tmp = sm_p.tile([64, 512], BF, tag="cno", name=_nm("cno"))
                            nc.vector.tensor_mul(tmp[:], cxp[0:64, :], bcs[:])
                            a = g * 2 + hp
                            nc.sync.dma_start(
                                out=ctxt[0:64, a, cc * 256:(cc + 1) * 256],
                                in_=tmp[:, 0:256])
                            nc.sync.dma_start(
                                out=ctxt[64:128, a, cc * 256:(cc + 1) * 256],
                                in_=tmp[:, 256:512])

            # ========== PHASE C: out-proj + residual + rmsnorm2 ==========
            with contextlib.ExitStack() as pc:
                xo_p = pc.enter_context(tc.tile_pool(name="xoC", bufs=1))
                wo_p = pc.enter_context(tc.tile_pool(name="woC", bufs=24))
                tmp2_p = pc.enter_context(tc.tile_pool(name="tmpC", bufs=4))
                ibc2_p = pc.enter_context(
                    tc.tile_pool(name="ibc2C", bufs=1, space="PSUM"))
                ps_y = pc.enter_context(
                    tc.tile_pool(name="psyC", bufs=4, space="PSUM"))
                ps_s2 = pc.enter_context(
                    tc.tile_pool(name="pss2", bufs=1, space="PSUM"))

                xo = xo_p.tile([P, KD, 512], BF, tag="xo", name=_nm("xo"))
                for k in range(KD):
                    nc.sync.dma_start(
                        out=xo[:, k, 0:256], in_=xT.ap()[k * P:(k + 1) * P, 0:256])
                    nc.sync.dma_start(
                        out=xo[:, k, 256:512],
                        in_=xT.ap()[k * P:(k + 1) * P, R_SMALL:R_SMALL + 256])

                for mg in range(8):
                    yps = [ps_y.tile([P, 512], F32, tag="y", name=_nm("y"))
                           for _ in range(2)]
                    for k in range(KD):
                        wo_t = wo_p.tile([P, 256], BF, tag="wo", name=_nm("wo"))
                        nc.sync.dma_start(
                            out=wo_t,
                            in_=w_o.ap()[k * P:(k + 1) * P,
                                         mg * 256:(mg + 1) * 256])
                        for mi in range(2):
                            nc.tensor.matmul(
                                yps[mi], wo_t[:, mi * P:(mi + 1) * P],
                                ctxt[:, k, :],
                                start=(k == 0), stop=(k == KD - 1))
                    for mi in range(2):
                        m = mg * 2 + mi
                        nc.vector.tensor_add(yT[:, m, :], yps[mi][:], xo[:, m, :])

                ss2 = ps_s2.tile([1, 512], F32, tag="ss2", name=_nm("ss2"))
                for m in range(KD):
                    sq2 = tmp2_p.tile([P, 512], F32R, tag="sq2", name=_nm("sq2"))
                    nc.scalar.activation(sq2[:], yT[:, m, :], AF.Square)
                    nc.tensor.matmul(ss2[:], ones_sb[:], sq2[:],
                                     start=(m == 0), stop=(m == KD - 1))
                std2 = tmp2_p.tile([1, 512], F32, tag="std2", name=_nm("std2"))
                nc.scalar.activation(std2[:], ss2[:], AF.Sqrt,
                                     bias=eps_sb[0:1, :], scale=INV_D)
                inv2 = tmp2_p.tile([1, 512], F32, tag="inv2", name=_nm("inv2"))
                nc.vector.reciprocal(inv2[:], std2[:])
                ibc2 = ibc2_p.tile([P, 512], F32, tag="ibc2", name=_nm("ibc2"))
                nc.tensor.matmul(ibc2[:], onesR_sb[:], inv2[:],
                                 start=True, stop=True)
                for m in range(KD):
                    nc.vector.scalar_tensor_tensor(
                        h2[:, m, :], yT[:, m, :], w2_sb[:, m:m + 1], ibc2[:],
                        AluOpType.mult, AluOpType.mult)

            pbc.close()

            # ========== PHASE D: SwiGLU MLP ==========
            with contextlib.ExitStack() as pd:
                ht_p = pd.enter_context(tc.tile_pool(name="htD", bufs=32))
                y2_p = pd.enter_context(tc.tile_pool(name="y2D", bufs=1))
                wgu_p = pd.enter_context(tc.tile_pool(name="wguD", bufs=32))
                wd_p = pd.enter_context(tc.tile_pool(name="wdD", bufs=24))
                sil_p = pd.enter_context(tc.tile_pool(name="silD", bufs=4))
                o_p = pd.enter_context(tc.tile_pool(name="oD", bufs=4))
                ps_gu = pd.enter_context(
                    tc.tile_pool(name="psgu", bufs=6, space="PSUM"))
                ps_d = pd.enter_context(
                    tc.tile_pool(name="psd", bufs=2, space="PSUM"))

                y2acc = y2_p.tile([P, KD, 512], F32, tag="y2", name=_nm("y2"))
                for grp in range(4):
                    f0 = grp * 2048
                    hts = []
                    for fg in range(8):
                        gps = [ps_gu.tile([P, 512], F32, tag="gu", name=_nm("gu"))
                               for _ in range(2)]
                        ups = [ps_gu.tile([P, 512], F32, tag="gu", name=_nm("gu"))
                               for _ in range(2)]
                        for k in range(KD):
                            wg_t = wgu_p.tile([P, 256], BF, tag="wg", name=_nm("wg"))
                            nc.sync.dma_start(
                                out=wg_t,
                                in_=w_g.ap()[k * P:(k + 1) * P,
                                             f0 + fg * 256:f0 + (fg + 1) * 256])
                            wu_t = wgu_p.tile([P, 256], BF, tag="wu", name=_nm("wu"))
                            nc.sync.dma_start(
                                out=wu_t,
                                in_=w_u.ap()[k * P:(k + 1) * P,
                                             f0 + fg * 256:f0 + (fg + 1) * 256])
                            for mi in range(2):
                                nc.tensor.matmul(
                                    gps[mi], wg_t[:, mi * P:(mi + 1) * P],
                                    h2[:, k, :],
                                    start=(k == 0), stop=(k == KD - 1))
                                nc.tensor.matmul(
                                    ups[mi], wu_t[:, mi * P:(mi + 1) * P],
                                    h2[:, k, :],
                                    start=(k == 0), stop=(k == KD - 1))
                        for mi in range(2):
                            sil = sil_p.tile([P, 512], F32, tag="sil", name=_nm("sil"))
                            nc.scalar.activation(sil[:], gps[mi][:], AF.Silu)
                            ht = ht_p.tile([P, 512], BF, tag="ht", name=_nm("ht"))
                            nc.vector.tensor_mul(ht[:], sil[:], ups[mi][:])
                            hts.append(ht)
                    for mg in range(8):
                        dps = [ps_d.tile([P, 512], F32, tag="d", name=_nm("d"))
                               for _ in range(2)]
                        for kk in range(16):
                            kr = f0 + kk * P
                            wd_t = wd_p.tile([P, 256], BF, tag="wd", name=_nm("wd"))
                            nc.sync.dma_start(
                                out=wd_t,
                                in_=w_d.ap()[kr:kr + P, mg * 256:(mg + 1) * 256])
                            for mi in range(2):
                                nc.tensor.matmul(
                                    dps[mi], wd_t[:, mi * P:(mi + 1) * P],
                                    hts[kk][:],
                                    start=(kk == 0), stop=(kk == 15))
                        for mi in range(2):
                            m = mg * 2 + mi
                            if grp == 0:
                                nc.vector.tensor_copy(y2acc[:, m, :], dps[mi][:])
                            else:
                                nc.vector.tensor_add(
                                    y2acc[:, m, :], y2acc[:, m, :], dps[mi][:])

                for m in range(KD):
                    o = o_p.tile([P, 512], F32, tag="o", name=_nm("o"))
                    nc.vector.tensor_add(o[:], y2acc[:, m, :], yT[:, m, :])
                    nc.sync.dma_start(
                        out=outT.ap()[m * P:(m + 1) * P, :], in_=o)

    nc.compile()
    return nc


# ======================= host-side prep =======================

def _host_prep_const(w_qkv, w_out, w_gate, w_up, w_down, w_norm1, w_norm2):
    """Core-independent tensors (computed once, shared across cores)."""
    f32 = np.float32
    # diagonal causal masks, duplicated for the paired heads (cols 0:256 and
    # 256:512 are the same 256 queries for two different heads)
    m = np.zeros((2, P, 512), dtype=f32)
    j = np.arange(256)[None, :]
    k_ = np.arange(P)[:, None]
    m[0, :, 0:256] = np.where(k_ > j, NEG, 0.0)
    m[0, :, 256:512] = m[0, :, 0:256]
    m[1, :, 0:256] = np.where(k_ + P > j, NEG, 0.0)
    m[1, :, 256:512] = m[1, :, 0:256]

    perm = np.zeros((P, P), dtype=f32)
    for r in range(P):
        d = r % 64
        s = r + 32 if d < 32 else r - 32
        perm[s, r] = 1.0

    return {
        "maskd": m,
        "w_q": np.ascontiguousarray(w_qkv[:, :2048]).astype(BF16),
        "w_k": np.ascontiguousarray(w_qkv[:, 2048:2560]).astype(BF16),
        "w_v": np.ascontiguousarray(w_qkv[:, 2560:3072]).astype(BF16),
        "w_o": np.asarray(w_out).astype(BF16),
        "w_g": np.asarray(w_gate).astype(BF16),
        "w_u": np.asarray(w_up).astype(BF16),
        "w_d": np.asarray(w_down).astype(BF16),
        "w_n1": np.asarray(w_norm1, dtype=f32),
        "w_n2": np.asarray(w_norm2, dtype=f32),
        "permM": perm,
        "onesC": np.ones((P, 1), dtype=f32),
        "onesR": np.ones((1, P), dtype=f32),
    }


def _host_prep_core(c, x, shared):
    """Per-core layout/slicing + rope tables."""
    f32 = np.float32
    if c <= 3:
        b_small, ch_small = 0, c
        b_large, ch_large = 1, 7 - c
    else:
        b_small, ch_small = 1, 7 - c
        b_large, ch_large = 0, c

    xT_full0 = x[b_small].T  # [D, S]
    xT_full1 = x[b_large].T

    xTc = np.zeros((D_MODEL, N_KVCOL), dtype=f32)
    pos = np.zeros(N_KVCOL, dtype=np.int64)
    # small region: [own | prefix | pad]
    o0 = ch_small * CHUNK
    xTc[:, 0:CHUNK] = xT_full0[:, o0:o0 + CHUNK]
    pos[0:CHUNK] = np.arange(o0, o0 + CHUNK)
    npre = o0
    xTc[:, CHUNK:CHUNK + npre] = xT_full0[:, 0:npre]
    pos[CHUNK:CHUNK + npre] = np.arange(npre)
    # large region
    o1 = ch_large * CHUNK
    xTc[:, R_SMALL:R_SMALL + CHUNK] = xT_full1[:, o1:o1 + CHUNK]
    pos[R_SMALL:R_SMALL + CHUNK] = np.arange(o1, o1 + CHUNK)
    npre1 = o1
    xTc[:, R_SMALL + CHUNK:R_SMALL + CHUNK + npre1] = xT_full1[:, 0:npre1]
    pos[R_SMALL + CHUNK:R_SMALL + CHUNK + npre1] = np.arange(npre1)

    # rope tables, replicated for 2 heads per 128 partitions, sign folded
    inv_freq = (ROPE_BASE ** (-np.arange(0, HEAD_DIM, 2, dtype=np.float64)
                              / HEAD_DIM))  # [32]
    ang = pos[None, :] * inv_freq[:, None]          # [32, N_KVCOL]
    cos32 = np.cos(ang)
    sin32 = np.sin(ang)
    cosT = np.empty((P, N_KVCOL), dtype=f32)
    sinT = np.empty((P, N_KVCOL), dtype=f32)
    for hh in range(2):
        r = hh * 64
        cosT[r:r + 32] = cos32
        cosT[r + 32:r + 64] = cos32
        sinT[r:r + 32] = -sin32
        sinT[r + 32:r + 64] = sin32

    # gate column: 1.0 for real key-tiles, 0.0 for padding
    vgate = np.zeros((2, 16, P), dtype=f32)
    vgate[0, :2 + 2 * ch_small, :] = 1.0
    vgate[1, :2 + 2 * ch_large, :] = 1.0

    d = {
        "xT": np.ascontiguousarray(xTc).astype(BF16),
        "cosT": cosT.astype(BF16), "sinT": sinT.astype(BF16),
        "vgate": vgate.astype(BF16),
    }
    d.update(shared)
    return d


def run(inputs, trace=False):
    if "nc" not in _prog_cache:
        _prog_cache["nc"] = _build_program()
    nc = _prog_cache["nc"]
    from concourse.bass_utils import run_bass_kernel_spmd

    shared = _host_prep_const(
        np.asarray(inputs["w_qkv"]), np.asarray(inputs["w_out"]),
        np.asarray(inputs["w_gate"]), np.asarray(inputs["w_up"]),
        np.asarray(inputs["w_down"]), np.asarray(inputs["w_norm1"]),
        np.asarray(inputs["w_norm2"]))
    x = np.asarray(inputs["x"])
    in_maps = [_host_prep_core(c, x, shared) for c in range(N_CORES)]
    res = run_bass_kernel_spmd(nc, in_maps, core_ids=list(range(N_CORES)),
                               trace=trace)

    out = np.empty((B, S, D_MODEL), dtype=np.float32)
    for c in range(N_CORES):
        oT = res.results[c]["outT"]  # [D, 512]
        if c <= 3:
            b_small, ch_small = 0, c
            b_large, ch_large = 1, 7 - c
        else:
            b_small, ch_small = 1, 7 - c
            b_large, ch_large = 0, c
        out[b_small, ch_small * CHUNK:(ch_small + 1) * CHUNK] = oT[:, 0:256].T
        out[b_large, ch_large * CHUNK:(ch_large + 1) * CHUNK] = oT[:, 256:512].T
    return out, res


def kernel(**inputs):
    out, _ = run(inputs, trace=False)
    return out


# revision 18
# speedup vs baseline: 1.4194x; 1.0189x over previous
"""Llama MHA layer on 8 TRN2 NeuronCores.

Sharding: causal-balanced sequence sharding, no collectives. Core c owns
batch-0 chunk c and batch-1 chunk 7-c (256 tokens each). Each core
recomputes K/V projections for its chunks' prefixes locally. Per-core
KV token columns are laid out [own | prefix | zero-pad] in two fixed-size
regions (1024 / 2048 cols) so the SPMD program is identical on all cores;
padding key-tiles are neutralized by a data-driven gate column fused into
the attention V matmul (which also computes the softmax denominator).

v2: weights/activations in bf16 (halves HBM traffic so weight streams
never starve the PE), rms-norm folded into the activations via a fused
scalar_tensor_tensor (plain cos/sin tables, no per-block scaling), gpsimd
partition_broadcast instead of DRAM bounces, attention head-pairing so
score/ctx matmuls run at free-dim 512, and block-pipelined emission so
the PE queue never sits behind an unmet dependency.
"""

import numpy as np
import ml_dtypes

BF16 = ml_dtypes.bfloat16

D_MODEL = 2048
N_HEADS = 32
N_KV = 8
HEAD_DIM = 64
D_FF = 8192
ROPE_BASE = 10000.0
EPS = 1e-5
B, S = 2, 2048
CHUNK = 256
P = 128
N_CORES = 8
R_SMALL = 1024   # cols in small-chunk region
R_LARGE = 2048   # cols in large-chunk region
N_KVCOL = R_SMALL + R_LARGE   # 3072
NEG = -1e30
BLK = 512
NBLK = N_KVCOL // BLK  # 6
KD = D_MODEL // P      # 16

_prog_cache = {}


def _build_program():
    import concourse.bacc as bacc
    import concourse.bass as bass
    import concourse.mybir as mybir
    import concourse.tile as tile
    from concourse import library_config
    from concourse.alu_op_type import AluOpType

    F32 = mybir.dt.float32
    F32R = mybir.dt.float32r
    BF = mybir.dt.bfloat16
    AF = mybir.ActivationFunctionType

    nc = bacc.Bacc(None, target_bir_lowering=False)

    # ---- inputs -------------------------------------------------------
    xT = nc.dram_tensor("xT", [D_MODEL, N_KVCOL], BF, kind="ExternalInput")
    cosT = nc.dram_tensor("cosT", [P, N_KVCOL], BF, kind="ExternalInput")
    sinT = nc.dram_tensor("sinT", [P, N_KVCOL], BF, kind="ExternalInput")
    maskd = nc.dram_tensor("maskd", [2, P, 512], F32, kind="ExternalInput")
    vgate = nc.dram_tensor("vgate", [2, 16, P], BF, kind="ExternalInput")
    w_q = nc.dram_tensor("w_q", [D_MODEL, 2048], BF, kind="ExternalInput")
    w_k = nc.dram_tensor("w_k", [D_MODEL, 512], BF, kind="ExternalInput")
    w_v = nc.dram_tensor("w_v", [D_MODEL, 512], BF, kind="ExternalInput")
    w_o = nc.dram_tensor("w_o", [D_MODEL, D_MODEL], BF, kind="ExternalInput")
    w_g = nc.dram_tensor("w_g", [D_MODEL, D_FF], BF, kind="ExternalInput")
    w_u = nc.dram_tensor("w_u", [D_MODEL, D_FF], BF, kind="ExternalInput")
    w_d = nc.dram_tensor("w_d", [D_FF, D_MODEL], BF, kind="ExternalInput")
    w_n1 = nc.dram_tensor("w_n1", [D_MODEL], F32, kind="ExternalInput")
    w_n2 = nc.dram_tensor("w_n2", [D_MODEL], F32, kind="ExternalInput")
    permM = nc.dram_tensor("permM", [P, P], F32R, kind="ExternalInput")
    onesC = nc.dram_tensor("onesC", [P, 1], F32R, kind="ExternalInput")
    onesR = nc.dram_tensor("onesR", [1, P], F32, kind="ExternalInput")
    onesMd = nc.dram_tensor("onesMd", [P, P], F32R, kind="ExternalInput")
    outT = nc.dram_tensor("outT", [D_MODEL, 512], F32, kind="ExternalOutput")

    INV_D = 1.0 / D_MODEL
    ATT_SCALE = 1.0 / np.sqrt(HEAD_DIM)

    _name_ctr = [0]

    def _nm(tag):
        _name_ctr[0] += 1
        return f"{tag}_{_name_ctr[0]}"

    with tile.TileContext(nc) as tc:
        import contextlib
        stack = contextlib.ExitStack()
        with stack:
            dr = stack.enter_context(tc.tile_pool(name="dr", bufs=1, space="DRAM"))
            const = stack.enter_context(tc.tile_pool(name="const", bufs=1))

            QT_s = dr.tile([2048, 512], BF, tag="QT_s", name=_nm("QT_s"))
            KT_s = dr.tile([512, N_KVCOL], BF, tag="KT_s", name=_nm("KT_s"))
            V_s = dr.tile([N_KVCOL, 512], BF, tag="V_s", name=_nm("V_s"))

            ones_sb = const.tile([P, 1], F32R, tag="ones", name=_nm("ones"))
            nc.sync.dma_start(out=ones_sb, in_=onesC.ap())
            perm_sb = const.tile([P, P], F32R, tag="perm", name=_nm("perm"))
            nc.sync.dma_start(out=perm_sb, in_=permM.ap())
            w1_sb = const.tile([P, KD], F32, tag="w1", name=_nm("w1"))
            nc.sync.dma_start(out=w1_sb, in_=w_n1.ap().rearrange("(k p) -> p k", p=P))
            w2_sb = const.tile([P, KD], F32, tag="w2", name=_nm("w2"))
            nc.sync.dma_start(out=w2_sb, in_=w_n2.ap().rearrange("(k p) -> p k", p=P))
            mask_sb = const.tile([P, 2, 512], F32, tag="mask", name=_nm("mask"))
            nc.sync.dma_start(out=mask_sb, in_=maskd.ap().rearrange("t p n -> p t n"))
            eps_sb = const.tile([P, 1], F32, tag="eps", name=_nm("eps"))
            nc.vector.memset(eps_sb, EPS)
            onesR_sb = const.tile([1, P], F32, tag="onesR", name=_nm("onesR"))
            nc.sync.dma_start(out=onesR_sb, in_=onesR.ap())
            onesM = const.tile([P, P], F32R, tag="onesM", name=_nm("onesM"))
            nc.sync.dma_start(out=onesM, in_=onesMd.ap())

            # ========== PHASE A: rmsnorm1 + QKV proj + rope ==========
            with contextlib.ExitStack() as pa:
                tabs_p = pa.enter_context(tc.tile_pool(name="tabsA", bufs=1))
                wres_p = pa.enter_context(tc.tile_pool(name="wresA", bufs=1))
                xr_p = pa.enter_context(tc.tile_pool(name="xrA", bufs=28))
                sq_p = pa.enter_context(tc.tile_pool(name="sqA", bufs=4))
                xw_p = pa.enter_context(tc.tile_pool(name="xwA", bufs=2))
                st_p = pa.enter_context(tc.tile_pool(name="stA", bufs=4))
                ibc_p = pa.enter_context(
                    tc.tile_pool(name="ibcA", bufs=2, space="PSUM"))
                wq_p = pa.enter_context(tc.tile_pool(name="wqA", bufs=16))
                rop_p = pa.enter_context(tc.tile_pool(name="ropA", bufs=5))
                vt_p = pa.enter_context(tc.tile_pool(name="vtA", bufs=4))
                ps_ss = pa.enter_context(
                    tc.tile_pool(name="psssA", bufs=1, space="PSUM"))
                ps_mm = pa.enter_context(
                    tc.tile_pool(name="psmmA", bufs=3, space="PSUM"))
                ps_rp = pa.enter_context(
                    tc.tile_pool(name="psrpA", bufs=2, space="PSUM"))

                cos_sb = tabs_p.tile([P, N_KVCOL], BF, tag="cos", name=_nm("cos"))
                nc.sync.dma_start(out=cos_sb, in_=cosT.ap())
                sin_sb = tabs_p.tile([P, N_KVCOL], BF, tag="sin", name=_nm("sin"))
                nc.sync.dma_start(out=sin_sb, in_=sinT.ap())
                wk_sb = wres_p.tile([P, KD, 512], BF, tag="wk", name=_nm("wk"))
                nc.sync.dma_start(
                    out=wk_sb, in_=w_k.ap().rearrange("(k p) n -> p k n", p=P))
                wv_sb = wres_p.tile([P, KD, 512], BF, tag="wv", name=_nm("wv"))
                nc.sync.dma_start(
                    out=wv_sb, in_=w_v.ap().rearrange("(k p) n -> p k n", p=P))

                xws = [None, None]
                xrs_blk = {}

                def stage_S(b):
                    c0 = b * BLK
                    xrs = []
                    ssum = ps_ss.tile([P, BLK], F32, tag="ss", name=_nm("ss"))
                    for k in range(KD):
                        xr = xr_p.tile([P, BLK], BF, tag="xr", name=_nm("xr"))
                        nc.sync.dma_start(
                            out=xr, in_=xT.ap()[k * P:(k + 1) * P, c0:c0 + BLK])
                        xrs.append(xr)
                        sq = sq_p.tile([P, BLK], F32R, tag="sq", name=_nm("sq"))
                        if k % 2 == 0:
                            nc.scalar.activation(sq[:], xr[:], AF.Square)
                        else:
                            nc.vector.tensor_mul(sq[:], xr[:], xr[:])
                        nc.tensor.matmul(
                            ssum[:], onesM[:], sq[:],
                            start=(k == 0), stop=(k == KD - 1))
                    std = st_p.tile([1, BLK], F32, tag="std", name=_nm("std"))
                    nc.scalar.activation(
                        std[:], ssum[0:1, :], AF.Sqrt, bias=eps_sb[0:1, :],
                        scale=INV_D)
                    inv = st_p.tile([1, BLK], F32, tag="inv", name=_nm("inv"))
                    nc.vector.reciprocal(inv[:], std[:])
                    ibc = ibc_p.tile([P, BLK], F32, tag="ibc", name=_nm("ibc"))
                    nc.tensor.matmul(ibc[:], onesR_sb[:], inv[:],
                                     start=True, stop=True)
                    xw = xw_p.tile([P, KD, BLK], BF, tag="xw", name=_nm("xw"))
                    for k in range(KD):
                        nc.vector.scalar_tensor_tensor(
                            xw[:, k, :], xrs[k][:], w1_sb[:, k:k + 1], ibc[:],
                            AluOpType.mult, AluOpType.mult)
                    xws[b % 2] = xw
                    xrs_blk[b] = xrs

                def rope_spill(psum, w, cos_ap, sin_ap, dst_ap):
                    raw = rop_p.tile([P, 512], F32R, tag="rraw", name=_nm("rraw"))[:, :w]
                    nc.scalar.activation(raw, psum, AF.Copy)
                    rot = ps_rp.tile([P, 512], F32, tag="rot", name=_nm("rot"))[:, :w]
                    nc.tensor.matmul(rot, perm_sb[:], raw, start=True, stop=True)
                    t1 = rop_p.tile([P, 512], F32, tag="t1", name=_nm("t1"))[:, :w]
                    nc.vector.tensor_mul(t1, raw, cos_ap)
                    t2 = rop_p.tile([P, 512], F32, tag="t2", name=_nm("t2"))[:, :w]
                    nc.vector.tensor_mul(t2, rot, sin_ap)
                    fin = rop_p.tile([P, 512], BF, tag="fin", name=_nm("fin"))[:, :w]
                    nc.vector.tensor_add(fin, t1, t2)
                    nc.sync.dma_start(out=dst_ap, in_=fin)

                def stage_P(b):
                    c0 = b * BLK
                    xw = xws[b % 2]
                    # K projection (+rope) -> KT_s[512, c0:c0+512]
                    for m in range(4):
                        kps = ps_mm.tile([P, 512], F32, tag="mm", name=_nm("mm"))
                        for k in range(KD):
                            nc.tensor.matmul(
                                kps[:], wk_sb[:, k, m * P:(m + 1) * P],
                                xw[:, k, :],
                                start=(k == 0), stop=(k == KD - 1))
                        rope_spill(kps[:], 512,
                                   cos_sb[:, c0:c0 + 512], sin_sb[:, c0:c0 + 512],
                                   KT_s[m * P:(m + 1) * P, c0:c0 + 512])
                    # V projection -> V_s[c0:c0+512, :]
                    for tm in range(4):
                        vps = ps_mm.tile([P, 512], F32, tag="mm", name=_nm("mm"))
                        for k in range(KD):
                            nc.tensor.matmul(
                                vps[:], xw[:, k, tm * P:(tm + 1) * P],
                                wv_sb[:, k, :],
                                start=(k == 0), stop=(k == KD - 1))
                        vt = vt_p.tile([P, 512], BF, tag="vt", name=_nm("vt"))
                        nc.scalar.activation(vt[:], vps[:], AF.Copy)
                        nc.sync.dma_start(
                            out=V_s[c0 + tm * P:c0 + (tm + 1) * P, :], in_=vt)
                    # Q projection (blocks 0 and 2 hold own columns 0:256)
                    if b in (0, 2):
                        q0 = 0 if b == 0 else 256
                        for mg in range(8):
                            qps = [ps_mm.tile([P, 512], F32, tag="mm",
                                              name=_nm("mm"))[:, :256]
                                   for _ in range(2)]
                            for k in range(KD):
                                wq_t = wq_p.tile([P, 256], BF, tag="wq",
                                                 name=_nm("wq"))
                                nc.sync.dma_start(
                                    out=wq_t,
                                    in_=w_q.ap()[k * P:(k + 1) * P,
                                                 mg * 256:(mg + 1) * 256])
                                for mi in range(2):
                                    nc.tensor.matmul(
                                        qps[mi],
                                        wq_t[:, mi * P:(mi + 1) * P],
                                        xw[:, k, 0:256],
                                        start=(k == 0), stop=(k == KD - 1))
                            for mi in range(2):
                                m = mg * 2 + mi
                                rope_spill(
                                    qps[mi], 256,
                                    cos_sb[:, c0:c0 + 256], sin_sb[:, c0:c0 + 256],
                                    QT_s[m * P:(m + 1) * P, q0:q0 + 256])
                    del xrs_blk[b]

                # pipelined emission: norm-stage one block ahead of proj-stage
                stage_S(0)
                stage_S(1)
                for b in range(NBLK):
                    stage_P(b)
                    if b + 2 < NBLK:
                        stage_S(b + 2)

            # ========== PHASE B: attention ==========
            res_p = stack.enter_context(tc.tile_pool(name="res", bufs=1))
            h2_p = stack.enter_context(tc.tile_pool(name="h2", bufs=1))
            yT = res_p.tile([P, KD, 512], F32, tag="yT", name=_nm("yT"))
            h2 = h2_p.tile([P, KD, 512], BF, tag="h2", name=_nm("h2"))
            pbc = contextlib.ExitStack()
            ctx_p = pbc.enter_context(tc.tile_pool(name="ctx", bufs=1))
            ctxt = ctx_p.tile([P, KD, 512], BF, tag="ctxt", name=_nm("ctxt"))
            with contextlib.ExitStack() as pb:
                kv_p = pb.enter_context(tc.tile_pool(name="kvB", bufs=2))
                va_p = pb.enter_context(tc.tile_pool(name="vaB", bufs=48))
                qh_p = pb.enter_context(tc.tile_pool(name="qhB", bufs=6))
                ex_p = pb.enter_context(tc.tile_pool(name="exB", bufs=6))
                sm_p = pb.enter_context(tc.tile_pool(name="smB", bufs=8))
                ps_sc = pb.enter_context(
                    tc.tile_pool(name="pssc", bufs=3, space="PSUM"))
                ps_cx = pb.enter_context(
                    tc.tile_pool(name="pscx", bufs=3, space="PSUM"))
                ps_bc = pb.enter_context(
                    tc.tile_pool(name="psbc", bufs=1, space="PSUM"))
                ps_jk = pb.enter_context(
                    tc.tile_pool(name="psjk", bufs=1, space="PSUM"))

                for g in range(N_KV):
                    ksb = kv_p.tile([P, N_KVCOL], BF, tag="ksb", name=_nm("ksb"))
                    nc.vector.memset(ksb[64:128, :], 0.0)
                    nc.sync.dma_start(
                        out=ksb[0:64, :], in_=KT_s[g * 64:(g + 1) * 64, :])
                    for cc in range(2):
                        nkt = 8 if cc == 0 else 16
                        kc0 = 0 if cc == 0 else R_SMALL
                        vaugs = []
                        for kt in range(nkt):
                            va = va_p.tile([P, 65], BF, tag="va", name=_nm("va"))
                            nc.sync.dma_start(
                                out=va[:, 0:64],
                                in_=V_s[kc0 + kt * P:kc0 + (kt + 1) * P,
                                        g * 64:(g + 1) * 64])
                            nc.sync.dma_start(
                                out=va[:, 64:65],
                                in_=vgate.ap()[cc, kt, :].rearrange(
                                    "(p o) -> p o", o=1))
                            vaugs.append(va)
                        for hp in range(2):
                            h0 = g * 4 + 2 * hp
                            qh2 = qh_p.tile([P, 512], BF, tag="qh", name=_nm("qh"))
                            nc.vector.memset(qh2[64:128, :], 0.0)
                            nc.sync.dma_start(
                                out=qh2[0:64, 0:256],
                                in_=QT_s[h0 * 64:(h0 + 1) * 64,
                                         cc * 256:(cc + 1) * 256])
                            nc.sync.dma_start(
                                out=qh2[0:64, 256:512],
                                in_=QT_s[(h0 + 1) * 64:(h0 + 2) * 64,
                                         cc * 256:(cc + 1) * 256])
                            cxp = ps_cx.tile([65, 512], F32, tag="cx", name=_nm("cx"))

                            exs = {}

                            def emit_sc(kt):
                                scp = ps_sc.tile([P, 512], F32, tag="sc",
                                                 name=_nm("sc"))
                                nc.tensor.matmul(
                                    scp[:],
                                    ksb[:, kc0 + kt * P:kc0 + (kt + 1) * P],
                                    qh2[:], start=True, stop=True)
                                if kt < 2:
                                    nc.vector.tensor_add(
                                        scp[:], scp[:], mask_sb[:, kt, :])
                                ex = ex_p.tile([P, 512], BF, tag="ex", name=_nm("ex"))
                                nc.scalar.activation(
                                    ex[:], scp[:], AF.Exp, scale=ATT_SCALE)
                                exs[kt] = ex

                            # scores run 2 key-tiles ahead of the ctx matmuls;
                            # a junk matmul per iteration keeps PE duty above
                            # the HAM re-throttle threshold while exp (ACT) is
                            # the true critical path
                            emit_sc(0)
                            if nkt > 1:
                                emit_sc(1)
                            for kt in range(nkt):
                                if kt + 2 < nkt:
                                    emit_sc(kt + 2)
                                jk = ps_jk.tile([P, 512], F32, tag="jk",
                                                name=_nm("jk"))
                                nc.tensor.matmul(
                                    jk[:], ksb[:, kc0:kc0 + P], qh2[:],
                                    start=True, stop=True)
                                nc.tensor.matmul(
                                    cxp[:], vaugs[kt][:], exs.pop(kt),
                                    start=(kt == 0), stop=(kt == nkt - 1))
                            rec = sm_p.tile([1, 512], F32, tag="rec", name=_nm("rec"))
                            nc.vector.reciprocal(rec[:], cxp[64:65, :])
                            bcr = ps_bc.tile([64, 512], F32, tag="bcr", name=_nm("bcr"))
                            nc.tensor.matmul(bcr[:], onesR_sb[:, 0:64], rec[:],
                                             start=True, stop=True)
                            bcs = sm_p.tile([64, 512], F32, tag="bcs", name=_nm("bcs"))
                            nc.vector.tensor_copy(bcs[:], bcr[:])
                            tmp = sm_p.tile([64, 512], BF, tag="cno", name=_nm("cno"))
                            nc.vector.tensor_mul(tmp[:], cxp[0:64, :], bcs[:])
                            a = g * 2 + hp
                            nc.sync.dma_start(
                                out=ctxt[0:64, a, cc * 256:(cc + 1) * 256],
                                in_=tmp[:, 0:256])
                            nc.sync.dma_start(
                                out=ctxt[64:128, a, cc * 256:(cc + 1) * 256],
                                in_=tmp[:, 256:512])

            # ========== PHASE C: out-proj + residual + rmsnorm2 ==========
            with contextlib.ExitStack() as pc:
                xo_p = pc.enter_context(tc.tile_pool(name="xoC", bufs=1))
                wo_p = pc.enter_context(tc.tile_pool(name="woC", bufs=24))
                tmp2_p = pc.enter_context(tc.tile_pool(name="tmpC", bufs=4))
                ibc2_p = pc.enter_context(
                    tc.tile_pool(name="ibc2C", bufs=1, space="PSUM"))
                ps_y = pc.enter_context(
                    tc.tile_pool(name="psyC", bufs=4, space="PSUM"))
                ps_s2 = pc.enter_context(
                    tc.tile_pool(name="pss2", bufs=1, space="PSUM"))

                xo = xo_p.tile([P, KD, 512], BF, tag="xo", name=_nm("xo"))
                for k in range(KD):
                    nc.sync.dma_start(
                        out=xo[:, k, 0:256], in_=xT.ap()[k * P:(k + 1) * P, 0:256])
                    nc.sync.dma_start(
                        out=xo[:, k, 256:512],
                        in_=xT.ap()[k * P:(k + 1) * P, R_SMALL:R_SMALL + 256])

                for mg in range(8):
                    yps = [ps_y.tile([P, 512], F32, tag="y", name=_nm("y"))
                           for _ in range(2)]
                    for k in range(KD):
                        wo_t = wo_p.tile([P, 256], BF, tag="wo", name=_nm("wo"))
                        nc.sync.dma_start(
                            out=wo_t,
                            in_=w_o.ap()[k * P:(k + 1) * P,
                                         mg * 256:(mg + 1) * 256])
                        for mi in range(2):
                            nc.tensor.matmul(
                                yps[mi], wo_t[:, mi * P:(mi + 1) * P],
                                ctxt[:, k, :],
                                start=(k == 0), stop=(k == KD - 1))
                    for mi in range(2):
                        m = mg * 2 + mi
                        nc.vector.tensor_add(yT[:, m, :], yps[mi][:], xo[:, m, :])

                ss2 = ps_s2.tile([P, 512], F32, tag="ss2", name=_nm("ss2"))
                for m in range(KD):
                    sq2 = tmp2_p.tile([P, 512], F32R, tag="sq2", name=_nm("sq2"))
                    nc.scalar.activation(sq2[:], yT[:, m, :], AF.Square)
                    nc.tensor.matmul(ss2[:], onesM[:], sq2[:],
                                     start=(m == 0), stop=(m == KD - 1))
                std2 = tmp2_p.tile([1, 512], F32, tag="std2", name=_nm("std2"))
                nc.scalar.activation(std2[:], ss2[0:1, :], AF.Sqrt,
                                     bias=eps_sb[0:1, :], scale=INV_D)
                inv2 = tmp2_p.tile([1, 512], F32, tag="inv2", name=_nm("inv2"))
                nc.vector.reciprocal(inv2[:], std2[:])
                ibc2 = ibc2_p.tile([P, 512], F32, tag="ibc2", name=_nm("ibc2"))
                nc.tensor.matmul(ibc2[:], onesR_sb[:], inv2[:],
                                 start=True, stop=True)
                for m in range(KD):
                    nc.vector.scalar_tensor_tensor(
                        h2[:, m, :], yT[:, m, :], w2_sb[:, m:m + 1], ibc2[:],
                        AluOpType.mult, AluOpType.mult)

            pbc.close()

            # ========== PHASE D: SwiGLU MLP ==========
            with contextlib.ExitStack() as pd:
                ht_p = pd.enter_context(tc.tile_pool(name="htD", bufs=32))
                y2_p = pd.enter_context(tc.tile_pool(name="y2D", bufs=1))
                wgu_p = pd.enter_context(tc.tile_pool(name="wguD", bufs=32))
                wd_p = pd.enter_context(tc.tile_pool(name="wdD", bufs=24))
                sil_p = pd.enter_context(tc.tile_pool(name="silD", bufs=4))
                o_p = pd.enter_context(tc.tile_pool(name="oD", bufs=4))
                ps_gu = pd.enter_context(
                    tc.tile_pool(name="psgu", bufs=6, space="PSUM"))
                ps_d = pd.enter_context(
                    tc.tile_pool(name="psd", bufs=2, space="PSUM"))

                y2acc = y2_p.tile([P, KD, 512], F32, tag="y2", name=_nm("y2"))
                for grp in range(4):
                    f0 = grp * 2048
                    hts = []
                    for fg in range(8):
                        gps = [ps_gu.tile([P, 512], F32, tag="gu", name=_nm("gu"))
                               for _ in range(2)]
                        ups = [ps_gu.tile([P, 512], F32, tag="gu", name=_nm("gu"))
                               for _ in range(2)]
                        for k in range(KD):
                            wg_t = wgu_p.tile([P, 256], BF, tag="wg", name=_nm("wg"))
                            nc.sync.dma_start(
                                out=wg_t,
                                in_=w_g.ap()[k * P:(k + 1) * P,
                                             f0 + fg * 256:f0 + (fg + 1) * 256])
                            wu_t = wgu_p.tile([P, 256], BF, tag="wu", name=_nm("wu"))
                            nc.sync.dma_start(
                                out=wu_t,
                                in_=w_u.ap()[k * P:(k + 1) * P,
                                             f0 + fg * 256:f0 + (fg + 1) * 256])
                            for mi in range(2):
                                nc.tensor.matmul(
                                    gps[mi], wg_t[:, mi * P:(mi + 1) * P],
                                    h2[:, k, :],
                                    start=(k == 0), stop=(k == KD - 1))
                                nc.tensor.matmul(
                                    ups[mi], wu_t[:, mi * P:(mi + 1) * P],
                                    h2[:, k, :],
                                    start=(k == 0), stop=(k == KD - 1))
                        for mi in range(2):
                            sil = sil_p.tile([P, 512], F32, tag="sil", name=_nm("sil"))
                            nc.scalar.activation(sil[:], gps[mi][:], AF.Silu)
                            ht = ht_p.tile([P, 512], BF, tag="ht", name=_nm("ht"))
                            nc.vector.tensor_mul(ht[:], sil[:], ups[mi][:])
                            hts.append(ht)
                    for mg in range(8):
                        dps = [ps_d.tile([P, 512], F32, tag="d", name=_nm("d"))
                               for _ in range(2)]
                        for kk in range(16):
                            kr = f0 + kk * P
                            wd_t = wd_p.tile([P, 256], BF, tag="wd", name=_nm("wd"))
                            nc.sync.dma_start(
                                out=wd_t,
                                in_=w_d.ap()[kr:kr + P, mg * 256:(mg + 1) * 256])
                            for mi in range(2):
                                nc.tensor.matmul(
                                    dps[mi], wd_t[:, mi * P:(mi + 1) * P],
                                    hts[kk][:],
                                    start=(kk == 0), stop=(kk == 15))
                        for mi in range(2):
                            m = mg * 2 + mi
                            if grp == 0:
                                nc.vector.tensor_copy(y2acc[:, m, :], dps[mi][:])
                            else:
                                nc.vector.tensor_add(
                                    y2acc[:, m, :], y2acc[:, m, :], dps[mi][:])

                for m in range(KD):
                    o = o_p.tile([P, 512], F32, tag="o", name=_nm("o"))
                    nc.vector.tensor_add(o[:], y2acc[:, m, :], yT[:, m, :])
                    nc.sync.dma_start(
                        out=outT.ap()[m * P:(m + 1) * P, :], in_=o)

    nc.compile()
    return nc


# ======================= host-side prep =======================

def _host_prep_const(w_qkv, w_out, w_gate, w_up, w_down, w_norm1, w_norm2):
    """Core-independent tensors (computed once, shared across cores)."""
    f32 = np.float32
    # diagonal causal masks, duplicated for the paired heads (cols 0:256 and
    # 256:512 are the same 256 queries for two different heads)
    m = np.zeros((2, P, 512), dtype=f32)
    j = np.arange(256)[None, :]
    k_ = np.arange(P)[:, None]
    m[0, :, 0:256] = np.where(k_ > j, NEG, 0.0)
    m[0, :, 256:512] = m[0, :, 0:256]
    m[1, :, 0:256] = np.where(k_ + P > j, NEG, 0.0)
    m[1, :, 256:512] = m[1, :, 0:256]

    perm = np.zeros((P, P), dtype=f32)
    for r in range(P):
        d = r % 64
        s = r + 32 if d < 32 else r - 32
        perm[s, r] = 1.0

    return {
        "maskd": m,
        "w_q": np.ascontiguousarray(w_qkv[:, :2048]).astype(BF16),
        "w_k": np.ascontiguousarray(w_qkv[:, 2048:2560]).astype(BF16),
        "w_v": np.ascontiguousarray(w_qkv[:, 2560:3072]).astype(BF16),
        "w_o": np.asarray(w_out).astype(BF16),
        "w_g": np.asarray(w_gate).astype(BF16),
        "w_u": np.asarray(w_up).astype(BF16),
        "w_d": np.asarray(w_down).astype(BF16),
        "w_n1": np.asarray(w_norm1, dtype=f32),
        "w_n2": np.asarray(w_norm2, dtype=f32),
        "permM": perm,
        "onesC": np.ones((P, 1), dtype=f32),
        "onesR": np.ones((1, P), dtype=f32),
        "onesMd": np.ones((P, P), dtype=f32),
    }


def _host_prep_core(c, x, shared):
    """Per-core layout/slicing + rope tables."""
    f32 = np.float32
    if c <= 3:
        b_small, ch_small = 0, c
        b_large, ch_large = 1, 7 - c
    else:
        b_small, ch_small = 1, 7 - c
        b_large, ch_large = 0, c

    xT_full0 = x[b_small].T  # [D, S]
    xT_full1 = x[b_large].T

    xTc = np.zeros((D_MODEL, N_KVCOL), dtype=f32)
    pos = np.zeros(N_KVCOL, dtype=np.int64)
    # small region: [own | prefix | pad]
    o0 = ch_small * CHUNK
    xTc[:, 0:CHUNK] = xT_full0[:, o0:o0 + CHUNK]
    pos[0:CHUNK] = np.arange(o0, o0 + CHUNK)
    npre = o0
    xTc[:, CHUNK:CHUNK + npre] = xT_full0[:, 0:npre]
    pos[CHUNK:CHUNK + npre] = np.arange(npre)
    # large region
    o1 = ch_large * CHUNK
    xTc[:, R_SMALL:R_SMALL + CHUNK] = xT_full1[:, o1:o1 + CHUNK]
    pos[R_SMALL:R_SMALL + CHUNK] = np.arange(o1, o1 + CHUNK)
    npre1 = o1
    xTc[:, R_SMALL + CHUNK:R_SMALL + CHUNK + npre1] = xT_full1[:, 0:npre1]
    pos[R_SMALL + CHUNK:R_SMALL + CHUNK + npre1] = np.arange(npre1)

    # rope tables, replicated for 2 heads per 128 partitions, sign folded
    inv_freq = (ROPE_BASE ** (-np.arange(0, HEAD_DIM, 2, dtype=np.float64)
                              / HEAD_DIM))  # [32]
    ang = pos[None, :] * inv_freq[:, None]          # [32, N_KVCOL]
    cos32 = np.cos(ang)
    sin32 = np.sin(ang)
    cosT = np.empty((P, N_KVCOL), dtype=f32)
    sinT = np.empty((P, N_KVCOL), dtype=f32)
    for hh in range(2):
        r = hh * 64
        cosT[r:r + 32] = cos32
        cosT[r + 32:r + 64] = cos32
        sinT[r:r + 32] = -sin32
        sinT[r + 32:r + 64] = sin32

    # gate column: 1.0 for real key-tiles, 0.0 for padding
    vgate = np.zeros((2, 16, P), dtype=f32)
    vgate[0, :2 + 2 * ch_small, :] = 1.0
    vgate[1, :2 + 2 * ch_large, :] = 1.0

    d = {
        "xT": np.ascontiguousarray(xTc).astype(BF16),
        "cosT": cosT.astype(BF16), "sinT": sinT.astype(BF16),
        "vgate": vgate.astype(BF16),
    }
    d.update(shared)
    return d


def run(inputs, trace=False):
    if "nc" not in _prog_cache:
        _prog_cache["nc"] = _build_program()
    nc = _prog_cache["nc"]
    from concourse.bass_utils import run_bass_kernel_spmd

    shared = _host_prep_const(
        np.asarray(inputs["w_qkv"]), np.asarray(inputs["w_out"]),
        np.asarray(inputs["w_gate"]), np.asarray(inputs["w_up"]),
        np.asarray(inputs["w_down"]), np.asarray(inputs["w_norm1"]),
        np.asarray(inputs["w_norm2"]))
    x = np.asarray(inputs["x"])
    in_maps = [_host_prep_core(c, x, shared) for c in range(N_CORES)]
    res = run_bass_kernel_spmd(nc, in_maps, core_ids=list(range(N_CORES)),
                               trace=trace)

    out = np.empty((B, S, D_MODEL), dtype=np.float32)
    for c in range(N_CORES):
        oT = res.results[c]["outT"]  # [D, 512]
        if c <= 3:
            b_small, ch_small = 0, c
            b_large, ch_large = 1, 7 - c
        else:
            b_small, ch_small = 1, 7 - c
            b_large, ch_large = 0, c
        out[b_small, ch_small * CHUNK:(ch_small + 1) * CHUNK] = oT[:, 0:256].T
        out[b_large, ch_large * CHUNK:(ch_large + 1) * CHUNK] = oT[:, 256:512].T
    return out, res


def kernel(**inputs):
    out, _ = run(inputs, trace=False)
    return out
